# revision 20
# baseline (speedup 1.0000x reference)
"""MergedQKVParallelLinearWithLora on 8 TRN2 NeuronCores.

Token-parallel across the 8 cores: each core computes 4096 tokens of the
full (T=32768, O=3072) output. Per core, per 512-token tile:

  shrink:  s~ = (1/64) * mask * (x8 @ (8*A)^T)   fp8e4 DoubleRow, K=2048
  main:    base bf16 matmul k-tile 0 (start=True)
         + lora expand+bias (one fp8 DoubleRow matmul, K=256: subtile0 =
           s~ rows vs 8*B columns, subtile1 = oh/8 rows vs 8*lora_bias
           rows, zero padded) accumulating into the same PSUM bank
         + base bf16 matmuls k-tiles 1..15
         + per-channel bias (DVE tensor_scalar_add at PSUM eviction)

W (bf16, 96KB/partition) stays fully SBUF-resident, so x streams exactly
once per dtype (bf16 for the base matmul, fp8 pair-layout for the shrink).
Tile-0 inputs and W are loaded through separate 4-ktile chunk tiles so the
first consumers wait on one chunk DMA, not the whole tensor. All
reshapes/transposes/dtype casts are host-side; fp8 scales are chosen so
every operand sits in e4m3's normal range (A,B,bias *8; s~ /8; the product
scales cancel).
"""

import numpy as np
import ml_dtypes

import concourse.mybir as mybir
import concourse.tile as tile
from concourse import bacc
from concourse.bass_utils import run_bass_kernel_spmd

T, D, QS, KVS, L, R = 32768, 2048, 2048, 512, 8, 16
O = QS + 2 * KVS          # 3072
NCORES = 8
TC = T // NCORES          # 4096 tokens per core
NT = 512                  # tokens per tile (matmul moving dim)
NTT = TC // NT            # 8 token tiles
NKT = D // 128            # 16 contraction k-tiles
NBLK = O // 128           # 24 output-channel blocks
WBLK = 8                  # blocks per sub-pass (3 sub-passes)
NPASS = NBLK // WBLK
NQ = 4                    # k-chunks per 16-ktile load (4 ktiles each)

F32 = mybir.dt.float32
BF16 = mybir.dt.bfloat16
FP8 = mybir.dt.float8e4
DR = mybir.MatmulPerfMode.DoubleRow
BF16NP = ml_dtypes.bfloat16
E4NP = ml_dtypes.float8_e4m3


def build_program(tc_tokens=TC):
    ntt = tc_tokens // NT
    nc = bacc.Bacc(None, target_bir_lowering=False, debug=False)

    x8d = nc.dram_tensor("x8d", [ntt, 128, NKT, NT], FP8, kind="ExternalInput")
    xbd = nc.dram_tensor("xbd", [ntt, 128, NKT, NT], BF16, kind="ExternalInput")
    wd = nc.dram_tensor("wd", [NPASS, 128, NKT, WBLK * 128], BF16,
                        kind="ExternalInput")
    a8d = nc.dram_tensor("a8d", [128, NKT, 384], FP8, kind="ExternalInput")
    bcld = nc.dram_tensor("bcld", [128, 2, O], FP8, kind="ExternalInput")
    m8d = nc.dram_tensor("m8d", [128, tc_tokens], FP8, kind="ExternalInput")
    oh8d = nc.dram_tensor("oh8d", [8, tc_tokens], FP8, kind="ExternalInput")
    bad = nc.dram_tensor("bad", [128, NBLK], F32, kind="ExternalInput")
    outT = nc.dram_tensor("outT", [O, tc_tokens], F32, kind="ExternalOutput")

    def slice_of(j):
        return 0 if j < QS // 128 else (1 if j < (QS + KVS) // 128 else 2)

    with tile.TileContext(nc) as tc:
        with tc.tile_pool(name="const", bufs=1) as const, \
             tc.tile_pool(name="x8p", bufs=2) as x8p, \
             tc.tile_pool(name="xbp", bufs=2) as xbp, \
             tc.tile_pool(name="psm", bufs=8, space="PSUM") as psm, \
             tc.tile_pool(name="op", bufs=4) as op:
            # A8 and W as separate 4-ktile chunk tiles: consumers then wait
            # on one chunk's DMA instead of the full tensor
            a8_c = [const.tile([128, 4, 384], FP8, tag=f"a8_{q}",
                               name=f"a8_{q}") for q in range(NQ)]
            w_c = [[const.tile([128, 4, WBLK * 128], BF16, tag=f"w{p}_{q}",
                               name=f"w{p}_{q}") for q in range(NQ)]
                   for p in range(NPASS)]
            dum_t = const.tile([128, NT], BF16, tag="dum")
            bcl_t = const.tile([128, 2, O], FP8, tag="bcl")
            m8_t = const.tile([128, tc_tokens], FP8, tag="m8")
            ba_t = const.tile([128, NBLK], F32, tag="ba")
            st_all = [const.tile([128, 2, tc_tokens], FP8, tag=f"st{s}",
                                 name=f"st{s}") for s in range(3)]
            def load_x8(tt):
                ts = []
                for q in range(NQ):
                    t = x8p.tile([128, 4, NT], FP8, tag=f"x8_{q}",
                                 name=f"x8_{tt}_{q}")
                    nc.sync.dma_start(out=t[:], in_=x8d[tt][:, 4 * q:4 * q + 4, :])
                    ts.append(t)
                return ts

            def load_xb(tt):
                ts = []
                for q in range(NQ):
                    t = xbp.tile([128, 4, NT], BF16, tag=f"xb_{q}",
                                 name=f"xb_{tt}_{q}")
                    nc.sync.dma_start(out=t[:], in_=xbd[tt][:, 4 * q:4 * q + 4, :])
                    ts.append(t)
                return ts

            # PE warm-up: dummy matmuls keep the HAM activity monitor busy
            # during the initial DMA latency so the first real matmuls run
            # at full clock
            nc.any.memset(dum_t[:], 0)
            wps = psm.tile([128, NT], F32, tag="ps", name="warm")
            for _ in range(8):
                nc.tensor.matmul(wps[:], dum_t[:, 0:128], dum_t[:],
                                 start=True, stop=True, skip_group_check=True)

            # ---- startup DMAs: everything on the sync queue, in strict
            # first-use order, so the critical first chunks aren't racing
            # other loads for the shared DMA engines
            x8_c, xb_c = [], []
            for q in range(3):
                t = x8p.tile([128, 4, NT], FP8, tag=f"x8_{q}", name=f"x8_0_{q}")
                nc.sync.dma_start(out=t[:], in_=x8d[0][:, 4 * q:4 * q + 4, :])
                x8_c.append(t)
                nc.sync.dma_start(out=a8_c[q][:],
                                  in_=a8d[:, 4 * q:4 * q + 4, :])
            nc.sync.dma_start(out=m8_t[:], in_=m8d[:])
            nc.sync.dma_start(out=w_c[0][0][:], in_=wd[0][:, 0:4, :])
            t = xbp.tile([128, 4, NT], BF16, tag="xb_0", name="xb_0_0")
            nc.sync.dma_start(out=t[:], in_=xbd[0][:, 0:4, :])
            xb_c.append(t)
            for q in range(3, NQ):
                t = x8p.tile([128, 4, NT], FP8, tag=f"x8_{q}", name=f"x8_0_{q}")
                nc.sync.dma_start(out=t[:], in_=x8d[0][:, 4 * q:4 * q + 4, :])
                x8_c.append(t)
                nc.sync.dma_start(out=a8_c[q][:],
                                  in_=a8d[:, 4 * q:4 * q + 4, :])
            nc.any.memset(st_all[0][:, 1, :], 0)
            nc.sync.dma_start(out=st_all[0][0:8, 1, :], in_=oh8d[:])
            nc.sync.dma_start(out=bcl_t[:], in_=bcld[:])
            for q in range(1, NQ):
                nc.sync.dma_start(out=w_c[0][q][:],
                                  in_=wd[0][:, 4 * q:4 * q + 4, :])
                t = xbp.tile([128, 4, NT], BF16, tag=f"xb_{q}", name=f"xb_0_{q}")
                nc.sync.dma_start(out=t[:], in_=xbd[0][:, 4 * q:4 * q + 4, :])
                xb_c.append(t)
            for q in range(NQ):
                nc.sync.dma_start(out=w_c[1][q][:],
                                  in_=wd[1][:, 4 * q:4 * q + 4, :])
            nc.sync.dma_start(out=ba_t[:], in_=bad[:])
            for s in (1, 2):
                nc.any.memset(st_all[s][:, 1, :], 0)
                nc.sync.dma_start(out=st_all[s][0:8, 1, :], in_=oh8d[:])
            for q in range(NQ):
                nc.sync.dma_start(out=w_c[2][q][:],
                                  in_=wd[2][:, 4 * q:4 * q + 4, :])

            for tt in range(ntt):
                tsl = slice(tt * NT, (tt + 1) * NT)
                # ---- shrink: fp8 DoubleRow, all 3 slices ----
                for s in range(3):
                    ps = psm.tile([128, NT], F32, tag="ps", name=f"shr{s}_{tt}")
                    for k in range(NKT // 2):
                        nc.tensor.matmul(
                            ps[:],
                            a8_c[k // 2][:, 2 * (k % 2):2 * (k % 2) + 2,
                                         s * 128:(s + 1) * 128],
                            x8_c[k // 2][:, 2 * (k % 2):2 * (k % 2) + 2, :],
                            start=(k == 0), stop=(k == NKT // 2 - 1),
                            perf_mode=DR, skip_group_check=True,
                        )
                    nc.vector.tensor_mul(st_all[s][:, 0, tsl], ps[:], m8_t[:, tsl])

                # prefetch next token tile while mains chew
                if tt + 1 < ntt:
                    x8_next = load_x8(tt + 1)
                    xb_next = load_xb(tt + 1)
                else:
                    x8_next = xb_next = None

                # ---- main: sub-passes of channel blocks. Each bank opens
                # with a base bf16 matmul (start=True), the fp8 DoubleRow
                # lora-expand accumulates behind it, then the remaining
                # base k-tiles drain the bank.
                def xb_i(i):
                    return xb_c[i // 4][:, i % 4, :]

                def run_group(p, jlist, out_dma=nc.gpsimd.dma_start):
                    pss = {}
                    for j in jlist:
                        blk = j - p * WBLK
                        ps = psm.tile([128, NT], F32, tag="ps", name=f"ps{j}_{tt}")
                        pss[j] = ps
                        nc.tensor.matmul(
                            ps[:],
                            w_c[p][0][:, 0, blk * 128:(blk + 1) * 128],
                            xb_i(0),
                            start=True, stop=False, skip_group_check=True,
                        )
                    for j in jlist:
                        s = slice_of(j)
                        nc.tensor.matmul(
                            pss[j][:],
                            bcl_t[:, :, j * 128:(j + 1) * 128],
                            st_all[s][:, :, tsl],
                            start=False, stop=False,
                            perf_mode=DR, skip_group_check=True,
                        )
                    for j in jlist:
                        blk = j - p * WBLK
                        for i in range(1, NKT):
                            nc.tensor.matmul(
                                pss[j][:],
                                w_c[p][i // 4][:, i % 4,
                                               blk * 128:(blk + 1) * 128],
                                xb_i(i),
                                start=False, stop=(i == NKT - 1),
                                skip_group_check=True,
                            )
                        o_t = op.tile([128, NT], F32, tag="o")
                        nc.vector.tensor_scalar_add(o_t[:], pss[j][:],
                                                    ba_t[:, j:j + 1])
                        out_dma(
                            out=outT[j * 128:(j + 1) * 128, tsl], in_=o_t[:])

                for p in range(NPASS):
                    js = list(range(p * WBLK, (p + 1) * WBLK))
                    if tt == ntt - 1 and p == NPASS - 1:
                        # split the final sub-pass so the last outputs'
                        # DMA drains earlier
                        run_group(p, js[:4])
                        run_group(p, js[4:6], out_dma=nc.sync.dma_start)
                        run_group(p, js[6:], out_dma=nc.sync.dma_start)
                    else:
                        run_group(p, js)
                if x8_next is not None:
                    x8_c, xb_c = x8_next, xb_next
    nc.compile()
    return nc


_nc_cache = {}


def _get_program(tc_tokens=TC):
    if tc_tokens not in _nc_cache:
        _nc_cache[tc_tokens] = build_program(tc_tokens)
    return _nc_cache[tc_tokens]


def make_in_maps(x, W_qkv, bias_qkv, lora_a_q, lora_a_k, lora_a_v,
                 lora_b_q, lora_b_k, lora_b_v,
                 lora_bias_q, lora_bias_k, lora_bias_v,
                 token_lora_indices, ncores=NCORES):
    x = np.asarray(x, np.float32)
    idx = np.asarray(token_lora_indices).astype(np.int64)
    tc_tokens = x.shape[0] // ncores
    ntt = tc_tokens // NT

    # W: (NPASS, 128, NKT, WBLK*128); [pi, p, i, m] = W_qkv[pi*1024+m, i*128+p]
    wd = np.ascontiguousarray(
        np.asarray(W_qkv, np.float32).reshape(NPASS, WBLK * 128, NKT, 128)
        .transpose(0, 3, 2, 1)).astype(BF16NP)
    # A8: (128, NKT, 384); [p, i, m] = 8 * A_stack[m, i*128+p]
    a_stack = np.concatenate([
        np.asarray(lora_a_q, np.float32).reshape(L * R, D),
        np.asarray(lora_a_k, np.float32).reshape(L * R, D),
        np.asarray(lora_a_v, np.float32).reshape(L * R, D)], axis=0)
    a8d = np.ascontiguousarray(
        (a_stack * 8.0).reshape(384, NKT, 128).transpose(2, 1, 0)).astype(E4NP)
    # bcl: (128, 2, O); [:,0,:] = 8*B^T rows (l*R+r), [:8,1,:] = 8*lora_bias
    bcomb = np.concatenate([
        np.asarray(lora_b_q, np.float32).transpose(0, 2, 1).reshape(L * R, QS),
        np.asarray(lora_b_k, np.float32).transpose(0, 2, 1).reshape(L * R, KVS),
        np.asarray(lora_b_v, np.float32).transpose(0, 2, 1).reshape(L * R, KVS)],
        axis=1)
    biasL = np.concatenate([
        np.asarray(lora_bias_q, np.float32),
        np.asarray(lora_bias_k, np.float32),
        np.asarray(lora_bias_v, np.float32)], axis=1)
    bcld = np.zeros((128, 2, O), np.float32)
    bcld[:, 0, :] = bcomb * 8.0
    bcld[:8, 1, :] = biasL * 8.0
    bcld = bcld.astype(E4NP)
    bad = np.ascontiguousarray(
        np.asarray(bias_qkv, np.float32).reshape(NBLK, 128).T)
    lane = np.arange(128) // R

    in_maps = []
    for c in range(ncores):
        sl = slice(c * tc_tokens, (c + 1) * tc_tokens)
        xc = x[sl]
        # (ntt, 128, NKT, NT); [tt, p, i, n] = x[tt*NT+n, i*128+p]
        xr = np.ascontiguousarray(
            xc.reshape(ntt, NT, NKT, 128).transpose(0, 3, 2, 1))
        idx_c = idx[sl]
        in_maps.append({
            "x8d": xr.astype(E4NP),
            "xbd": xr.astype(BF16NP),
            "wd": wd,
            "a8d": a8d,
            "bcld": bcld,
            "m8d": np.where(idx_c[None, :] == lane[:, None],
                            np.float32(1 / 64), np.float32(0)).astype(E4NP),
            "oh8d": np.where(idx_c[None, :] == np.arange(L)[:, None],
                             np.float32(0.125), np.float32(0)).astype(E4NP),
            "bad": bad,
        })
    return in_maps, tc_tokens


def kernel(x, W_qkv, bias_qkv, lora_a_q, lora_a_k, lora_a_v,
           lora_b_q, lora_b_k, lora_b_v,
           lora_bias_q, lora_bias_k, lora_bias_v,
           token_lora_indices):
    in_maps, tc_tokens = make_in_maps(
        x, W_qkv, bias_qkv, lora_a_q, lora_a_k, lora_a_v,
        lora_b_q, lora_b_k, lora_b_v,
        lora_bias_q, lora_bias_k, lora_bias_v, token_lora_indices)
    nc = _get_program(tc_tokens)
    res = None
    for attempt in range(3):
        try:
            res = run_bass_kernel_spmd(nc, in_maps, list(range(NCORES)))
            break
        except Exception:
            if attempt == 2:
                raise
    out = np.empty((T, O), np.float32)
    for c in range(NCORES):
        out[c * tc_tokens:(c + 1) * tc_tokens] = res.results[c]["outT"].T
    return out


# revision 21
# speedup vs baseline: 1.0047x; 1.0047x over previous
"""MergedQKVParallelLinearWithLora on 8 TRN2 NeuronCores.

Token-parallel across the 8 cores: each core computes 4096 tokens of the
full (T=32768, O=3072) output. Per core, per 512-token tile:

  shrink:  s~ = (1/64) * mask * (x8 @ (8*A)^T)   fp8e4 DoubleRow, K=2048
  main:    base bf16 matmul k-tile 0 (start=True)
         + lora expand+bias (one fp8 DoubleRow matmul, K=256: subtile0 =
           s~ rows vs 8*B columns, subtile1 = oh/8 rows vs 8*lora_bias
           rows, zero padded) accumulating into the same PSUM bank
         + base bf16 matmuls k-tiles 1..15
         + per-channel bias (DVE tensor_scalar_add at PSUM eviction)

W (bf16, 96KB/partition) stays fully SBUF-resident, so x streams exactly
once per dtype (bf16 for the base matmul, fp8 pair-layout for the shrink).
Tile-0 inputs and W are loaded through separate 4-ktile chunk tiles so the
first consumers wait on one chunk DMA, not the whole tensor. All
reshapes/transposes/dtype casts are host-side; fp8 scales are chosen so
every operand sits in e4m3's normal range (A,B,bias *8; s~ /8; the product
scales cancel).
"""

import numpy as np
import ml_dtypes

import concourse.mybir as mybir
import concourse.tile as tile
from concourse import bacc
from concourse.bass_utils import run_bass_kernel_spmd

T, D, QS, KVS, L, R = 32768, 2048, 2048, 512, 8, 16
O = QS + 2 * KVS          # 3072
NCORES = 8
TC = T // NCORES          # 4096 tokens per core
NT = 512                  # tokens per tile (matmul moving dim)
NTT = TC // NT            # 8 token tiles
NKT = D // 128            # 16 contraction k-tiles
NBLK = O // 128           # 24 output-channel blocks
WBLK = 8                  # blocks per sub-pass (3 sub-passes)
NPASS = NBLK // WBLK
NQ = 4                    # k-chunks per 16-ktile load (4 ktiles each)

F32 = mybir.dt.float32
BF16 = mybir.dt.bfloat16
FP8 = mybir.dt.float8e4
DR = mybir.MatmulPerfMode.DoubleRow
BF16NP = ml_dtypes.bfloat16
E4NP = ml_dtypes.float8_e4m3


def build_program(tc_tokens=TC):
    ntt = tc_tokens // NT
    nc = bacc.Bacc(None, target_bir_lowering=False, debug=False)

    x8d = nc.dram_tensor("x8d", [ntt, 128, NKT, NT], FP8, kind="ExternalInput")
    xbd = nc.dram_tensor("xbd", [ntt, 128, NKT, NT], BF16, kind="ExternalInput")
    wd = nc.dram_tensor("wd", [NPASS, 128, NKT, WBLK * 128], BF16,
                        kind="ExternalInput")
    a8d = nc.dram_tensor("a8d", [128, NKT, 384], FP8, kind="ExternalInput")
    bcld = nc.dram_tensor("bcld", [128, 2, O], FP8, kind="ExternalInput")
    m8d = nc.dram_tensor("m8d", [128, tc_tokens], FP8, kind="ExternalInput")
    oh8d = nc.dram_tensor("oh8d", [8, tc_tokens], FP8, kind="ExternalInput")
    bad = nc.dram_tensor("bad", [128, NBLK], F32, kind="ExternalInput")
    outT = nc.dram_tensor("outT", [O, tc_tokens], F32, kind="ExternalOutput")

    def slice_of(j):
        return 0 if j < QS // 128 else (1 if j < (QS + KVS) // 128 else 2)

    with tile.TileContext(nc) as tc:
        with tc.tile_pool(name="const", bufs=1) as const, \
             tc.tile_pool(name="x8p", bufs=2) as x8p, \
             tc.tile_pool(name="xbp", bufs=2) as xbp, \
             tc.tile_pool(name="psm", bufs=8, space="PSUM") as psm, \
             tc.tile_pool(name="op", bufs=4) as op:
            # A8 and W as separate 4-ktile chunk tiles: consumers then wait
            # on one chunk's DMA instead of the full tensor
            a8_c = [const.tile([128, 4, 384], FP8, tag=f"a8_{q}",
                               name=f"a8_{q}") for q in range(NQ)]
            w_c = [[const.tile([128, 4, WBLK * 128], BF16, tag=f"w{p}_{q}",
                               name=f"w{p}_{q}") for q in range(NQ)]
                   for p in range(NPASS)]
            dum_t = const.tile([128, NT], BF16, tag="dum")
            bcl_t = const.tile([128, 2, O], FP8, tag="bcl")
            m8_t = const.tile([128, tc_tokens], FP8, tag="m8")
            ba_t = const.tile([128, NBLK], F32, tag="ba")
            st_all = [const.tile([128, 2, tc_tokens], FP8, tag=f"st{s}",
                                 name=f"st{s}") for s in range(3)]
            def load_x8(tt):
                ts = []
                for q in range(NQ):
                    t = x8p.tile([128, 4, NT], FP8, tag=f"x8_{q}",
                                 name=f"x8_{tt}_{q}")
                    nc.sync.dma_start(out=t[:], in_=x8d[tt][:, 4 * q:4 * q + 4, :])
                    ts.append(t)
                return ts

            def load_xb(tt):
                ts = []
                for q in range(NQ):
                    t = xbp.tile([128, 4, NT], BF16, tag=f"xb_{q}",
                                 name=f"xb_{tt}_{q}")
                    nc.sync.dma_start(out=t[:], in_=xbd[tt][:, 4 * q:4 * q + 4, :])
                    ts.append(t)
                return ts

            # PE warm-up: dummy matmuls keep the HAM activity monitor busy
            # during the initial DMA latency so the first real matmuls run
            # at full clock
            nc.any.memset(dum_t[:], 0)
            wps = psm.tile([128, NT], F32, tag="ps", name="warm")
            for _ in range(8):
                nc.tensor.matmul(wps[:], dum_t[:, 0:128], dum_t[:],
                                 start=True, stop=True, skip_group_check=True)

            # ---- startup DMAs: everything on the sync queue, in strict
            # first-use order, so the critical first chunks aren't racing
            # other loads for the shared DMA engines
            x8_c, xb_c = [], []
            for q in range(3):
                t = x8p.tile([128, 4, NT], FP8, tag=f"x8_{q}", name=f"x8_0_{q}")
                nc.sync.dma_start(out=t[:], in_=x8d[0][:, 4 * q:4 * q + 4, :])
                x8_c.append(t)
                nc.sync.dma_start(out=a8_c[q][:],
                                  in_=a8d[:, 4 * q:4 * q + 4, :])
            nc.sync.dma_start(out=m8_t[:], in_=m8d[:])
            nc.sync.dma_start(out=w_c[0][0][:], in_=wd[0][:, 0:4, :])
            t = xbp.tile([128, 4, NT], BF16, tag="xb_0", name="xb_0_0")
            nc.sync.dma_start(out=t[:], in_=xbd[0][:, 0:4, :])
            xb_c.append(t)
            for q in range(3, NQ):
                t = x8p.tile([128, 4, NT], FP8, tag=f"x8_{q}", name=f"x8_0_{q}")
                nc.sync.dma_start(out=t[:], in_=x8d[0][:, 4 * q:4 * q + 4, :])
                x8_c.append(t)
                nc.sync.dma_start(out=a8_c[q][:],
                                  in_=a8d[:, 4 * q:4 * q + 4, :])
            nc.any.memset(st_all[0][:, 1, :], 0)
            nc.sync.dma_start(out=st_all[0][0:8, 1, :], in_=oh8d[:])
            nc.sync.dma_start(out=bcl_t[:], in_=bcld[:])
            for q in range(1, NQ):
                nc.sync.dma_start(out=w_c[0][q][:],
                                  in_=wd[0][:, 4 * q:4 * q + 4, :])
                t = xbp.tile([128, 4, NT], BF16, tag=f"xb_{q}", name=f"xb_0_{q}")
                nc.sync.dma_start(out=t[:], in_=xbd[0][:, 4 * q:4 * q + 4, :])
                xb_c.append(t)
            for q in range(NQ):
                nc.sync.dma_start(out=w_c[1][q][:],
                                  in_=wd[1][:, 4 * q:4 * q + 4, :])
            nc.sync.dma_start(out=ba_t[:], in_=bad[:])
            for s in (1, 2):
                nc.any.memset(st_all[s][:, 1, :], 0)
                nc.sync.dma_start(out=st_all[s][0:8, 1, :], in_=oh8d[:])
            for q in range(NQ):
                nc.sync.dma_start(out=w_c[2][q][:],
                                  in_=wd[2][:, 4 * q:4 * q + 4, :])

            for tt in range(ntt):
                tsl = slice(tt * NT, (tt + 1) * NT)
                # ---- shrink: fp8 DoubleRow, all 3 slices ----
                for s in range(3):
                    ps = psm.tile([128, NT], F32, tag="ps", name=f"shr{s}_{tt}")
                    for k in range(NKT // 2):
                        nc.tensor.matmul(
                            ps[:],
                            a8_c[k // 2][:, 2 * (k % 2):2 * (k % 2) + 2,
                                         s * 128:(s + 1) * 128],
                            x8_c[k // 2][:, 2 * (k % 2):2 * (k % 2) + 2, :],
                            start=(k == 0), stop=(k == NKT // 2 - 1),
                            perf_mode=DR, skip_group_check=True,
                        )
                    nc.vector.tensor_mul(st_all[s][:, 0, tsl], ps[:], m8_t[:, tsl])

                # prefetch next token tile while mains chew
                if tt + 1 < ntt:
                    x8_next = load_x8(tt + 1)
                    xb_next = load_xb(tt + 1)
                else:
                    x8_next = xb_next = None

                # ---- main: sub-passes of channel blocks. Each bank opens
                # with a base bf16 matmul (start=True), the fp8 DoubleRow
                # lora-expand accumulates behind it, then the remaining
                # base k-tiles drain the bank.
                def xb_i(i):
                    return xb_c[i // 4][:, i % 4, :]

                def run_group(p, jlist, out_dma=nc.gpsimd.dma_start):
                    pss = {}
                    for j in jlist:
                        blk = j - p * WBLK
                        ps = psm.tile([128, NT], F32, tag="ps", name=f"ps{j}_{tt}")
                        pss[j] = ps
                        nc.tensor.matmul(
                            ps[:],
                            w_c[p][0][:, 0, blk * 128:(blk + 1) * 128],
                            xb_i(0),
                            start=True, stop=False, skip_group_check=True,
                        )
                    for j in jlist:
                        s = slice_of(j)
                        nc.tensor.matmul(
                            pss[j][:],
                            bcl_t[:, :, j * 128:(j + 1) * 128],
                            st_all[s][:, :, tsl],
                            start=False, stop=False,
                            perf_mode=DR, skip_group_check=True,
                        )
                    for j in jlist:
                        blk = j - p * WBLK
                        for i in range(1, NKT):
                            nc.tensor.matmul(
                                pss[j][:],
                                w_c[p][i // 4][:, i % 4,
                                               blk * 128:(blk + 1) * 128],
                                xb_i(i),
                                start=False, stop=(i == NKT - 1),
                                skip_group_check=True,
                            )
                        o_t = op.tile([128, NT], F32, tag="o")
                        nc.vector.tensor_scalar_add(o_t[:], pss[j][:],
                                                    ba_t[:, j:j + 1])
                        out_dma(
                            out=outT[j * 128:(j + 1) * 128, tsl], in_=o_t[:])

                for p in range(NPASS):
                    js = list(range(p * WBLK, (p + 1) * WBLK))
                    if tt == ntt - 1 and p == NPASS - 1:
                        # split the final sub-pass so the last outputs'
                        # DMA drains earlier
                        run_group(p, js[:4])
                        run_group(p, js[4:6], out_dma=nc.sync.dma_start)
                        run_group(p, js[6:], out_dma=nc.sync.dma_start)
                    else:
                        run_group(p, js)
                if x8_next is not None:
                    x8_c, xb_c = x8_next, xb_next
    nc.compile()
    return nc


_nc_cache = {}


def _get_program(tc_tokens=TC):
    if tc_tokens not in _nc_cache:
        _nc_cache[tc_tokens] = build_program(tc_tokens)
    return _nc_cache[tc_tokens]


def make_in_maps(x, W_qkv, bias_qkv, lora_a_q, lora_a_k, lora_a_v,
                 lora_b_q, lora_b_k, lora_b_v,
                 lora_bias_q, lora_bias_k, lora_bias_v,
                 token_lora_indices, ncores=NCORES):
    x = np.asarray(x, np.float32)
    idx = np.asarray(token_lora_indices).astype(np.int64)
    tc_tokens = x.shape[0] // ncores
    ntt = tc_tokens // NT

    # W: (NPASS, 128, NKT, WBLK*128); [pi, p, i, m] = W_qkv[pi*1024+m, i*128+p]
    wd = np.ascontiguousarray(
        np.asarray(W_qkv, np.float32).reshape(NPASS, WBLK * 128, NKT, 128)
        .transpose(0, 3, 2, 1)).astype(BF16NP)
    # A8: (128, NKT, 384); [p, i, m] = 8 * A_stack[m, i*128+p]
    a_stack = np.concatenate([
        np.asarray(lora_a_q, np.float32).reshape(L * R, D),
        np.asarray(lora_a_k, np.float32).reshape(L * R, D),
        np.asarray(lora_a_v, np.float32).reshape(L * R, D)], axis=0)
    a8d = np.ascontiguousarray(
        (a_stack * 8.0).reshape(384, NKT, 128).transpose(2, 1, 0)).astype(E4NP)
    # bcl: (128, 2, O); [:,0,:] = 8*B^T rows (l*R+r), [:8,1,:] = 8*lora_bias
    bcomb = np.concatenate([
        np.asarray(lora_b_q, np.float32).transpose(0, 2, 1).reshape(L * R, QS),
        np.asarray(lora_b_k, np.float32).transpose(0, 2, 1).reshape(L * R, KVS),
        np.asarray(lora_b_v, np.float32).transpose(0, 2, 1).reshape(L * R, KVS)],
        axis=1)
    biasL = np.concatenate([
        np.asarray(lora_bias_q, np.float32),
        np.asarray(lora_bias_k, np.float32),
        np.asarray(lora_bias_v, np.float32)], axis=1)
    bcld = np.zeros((128, 2, O), np.float32)
    bcld[:, 0, :] = bcomb * 8.0
    bcld[:8, 1, :] = biasL * 8.0
    bcld = bcld.astype(E4NP)
    bad = np.ascontiguousarray(
        np.asarray(bias_qkv, np.float32).reshape(NBLK, 128).T)
    lane = np.arange(128) // R

    in_maps = []
    for c in range(ncores):
        sl = slice(c * tc_tokens, (c + 1) * tc_tokens)
        xc = x[sl]
        # (ntt, 128, NKT, NT); [tt, p, i, n] = x[tt*NT+n, i*128+p]
        xr = np.ascontiguousarray(
            xc.reshape(ntt, NT, NKT, 128).transpose(0, 3, 2, 1))
        idx_c = idx[sl]
        in_maps.append({
            "x8d": xr.astype(E4NP),
            "xbd": xr.astype(BF16NP),
            "wd": wd,
            "a8d": a8d,
            "bcld": bcld,
            "m8d": np.where(idx_c[None, :] == lane[:, None],
                            np.float32(1 / 64), np.float32(0)).astype(E4NP),
            "oh8d": np.where(idx_c[None, :] == np.arange(L)[:, None],
                             np.float32(0.125), np.float32(0)).astype(E4NP),
            "bad": bad,
        })
    return in_maps, tc_tokens


def kernel(x, W_qkv, bias_qkv, lora_a_q, lora_a_k, lora_a_v,
           lora_b_q, lora_b_k, lora_b_v,
           lora_bias_q, lora_bias_k, lora_bias_v,
           token_lora_indices):
    in_maps, tc_tokens = make_in_maps(
        x, W_qkv, bias_qkv, lora_a_q, lora_a_k, lora_a_v,
        lora_b_q, lora_b_k, lora_b_v,
        lora_bias_q, lora_bias_k, lora_bias_v, token_lora_indices)
    nc = _get_program(tc_tokens)
    res = None
    for attempt in range(3):
        try:
            res = run_bass_kernel_spmd(nc, in_maps, list(range(NCORES)))
            break
        except Exception:
            if attempt == 2:
                raise
    out = np.empty((tc_tokens * NCORES, O), np.float32)
    for c in range(NCORES):
        out[c * tc_tokens:(c + 1) * tc_tokens] = res.results[c]["outT"].T
    return out


# revision 22
# speedup vs baseline: 1.0590x; 1.0540x over previous
"""MergedQKVParallelLinearWithLora on 8 TRN2 NeuronCores.

Token-parallel across the 8 cores: each core computes 4096 tokens of the
full (T=32768, O=3072) output. Per core, per 512-token tile:

  shrink:  s~ = (1/64) * mask * (x8 @ (8*A)^T)   fp8e4 DoubleRow, K=2048
  main:    base bf16 matmul k-tile 0 (start=True)
         + lora expand+bias (one fp8 DoubleRow matmul, K=256: subtile0 =
           s~ rows vs 8*B columns, subtile1 = oh/8 rows vs 8*lora_bias
           rows, zero padded) accumulating into the same PSUM bank
         + base bf16 matmuls k-tiles 1..15
         + per-channel bias (DVE tensor_scalar_add at PSUM eviction)

W (bf16, 96KB/partition) stays fully SBUF-resident, so x streams exactly
once per dtype (bf16 for the base matmul, fp8 pair-layout for the shrink).
Tile-0 inputs and W are loaded through separate 4-ktile chunk tiles so the
first consumers wait on one chunk DMA, not the whole tensor. All
reshapes/transposes/dtype casts are host-side; fp8 scales are chosen so
every operand sits in e4m3's normal range (A,B,bias *8; s~ /8; the product
scales cancel).
"""

import numpy as np
import ml_dtypes

import concourse.mybir as mybir
import concourse.tile as tile
from concourse import bacc
from concourse.bass_utils import run_bass_kernel_spmd

T, D, QS, KVS, L, R = 32768, 2048, 2048, 512, 8, 16
O = QS + 2 * KVS          # 3072
NCORES = 8
TC = T // NCORES          # 4096 tokens per core
NT = 512                  # tokens per tile (matmul moving dim)
NTT = TC // NT            # 8 token tiles
NKT = D // 128            # 16 contraction k-tiles
NBLK = O // 128           # 24 output-channel blocks
WBLK = 8                  # blocks per sub-pass (3 sub-passes)
NPASS = NBLK // WBLK
NQ = 4                    # k-chunks per 16-ktile load (4 ktiles each)

F32 = mybir.dt.float32
BF16 = mybir.dt.bfloat16
FP8 = mybir.dt.float8e4
DR = mybir.MatmulPerfMode.DoubleRow
BF16NP = ml_dtypes.bfloat16
E4NP = ml_dtypes.float8_e4m3


def build_program(tc_tokens=TC):
    ntt = tc_tokens // NT
    nc = bacc.Bacc(None, target_bir_lowering=False, debug=False)

    x8d = nc.dram_tensor("x8d", [ntt, 128, NKT, NT], FP8, kind="ExternalInput")
    xbd = nc.dram_tensor("xbd", [ntt, 128, NKT, NT], BF16, kind="ExternalInput")
    wd = nc.dram_tensor("wd", [NPASS, 128, NKT, WBLK * 128], BF16,
                        kind="ExternalInput")
    a8d = nc.dram_tensor("a8d", [128, NKT, 384], FP8, kind="ExternalInput")
    w8d = nc.dram_tensor("w8d", [NPASS, 128, 2, WBLK * 128], FP8,
                         kind="ExternalInput")
    bcld = nc.dram_tensor("bcld", [128, 2, O], FP8, kind="ExternalInput")
    m8d = nc.dram_tensor("m8d", [128, tc_tokens], FP8, kind="ExternalInput")
    oh8d = nc.dram_tensor("oh8d", [8, tc_tokens], FP8, kind="ExternalInput")
    bad = nc.dram_tensor("bad", [128, NBLK], F32, kind="ExternalInput")
    outT = nc.dram_tensor("outT", [O, tc_tokens], F32, kind="ExternalOutput")

    def slice_of(j):
        return 0 if j < QS // 128 else (1 if j < (QS + KVS) // 128 else 2)

    with tile.TileContext(nc) as tc:
        with tc.tile_pool(name="const", bufs=1) as const, \
             tc.tile_pool(name="x8p", bufs=2) as x8p, \
             tc.tile_pool(name="xbp", bufs=2) as xbp, \
             tc.tile_pool(name="psm", bufs=8, space="PSUM") as psm, \
             tc.tile_pool(name="op", bufs=4) as op:
            # A8 and W as separate 4-ktile chunk tiles: consumers then wait
            # on one chunk's DMA instead of the full tensor
            a8_c = [const.tile([128, 4, 384], FP8, tag=f"a8_{q}",
                               name=f"a8_{q}") for q in range(NQ)]
            w_c = [[const.tile([128, 4, WBLK * 128], BF16, tag=f"w{p}_{q}",
                               name=f"w{p}_{q}") for q in range(NQ)]
                   for p in range(NPASS)]
            w8_c = [const.tile([128, 2, WBLK * 128], FP8, tag=f"w8_{p}",
                               name=f"w8_{p}") for p in range(NPASS)]
            dum_t = const.tile([128, NT], BF16, tag="dum")
            bcl_t = const.tile([128, 2, O], FP8, tag="bcl")
            m8_t = const.tile([128, tc_tokens], FP8, tag="m8")
            ba_t = const.tile([128, NBLK], F32, tag="ba")
            st_all = [const.tile([128, 2, tc_tokens], FP8, tag=f"st{s}",
                                 name=f"st{s}") for s in range(3)]
            def load_x8(tt):
                ts = []
                for q in range(NQ):
                    t = x8p.tile([128, 4, NT], FP8, tag=f"x8_{q}",
                                 name=f"x8_{tt}_{q}")
                    nc.sync.dma_start(out=t[:], in_=x8d[tt][:, 4 * q:4 * q + 4, :])
                    ts.append(t)
                return ts

            def load_xb(tt):
                ts = []
                for q in range(NQ):
                    t = xbp.tile([128, 4, NT], BF16, tag=f"xb_{q}",
                                 name=f"xb_{tt}_{q}")
                    nc.sync.dma_start(out=t[:], in_=xbd[tt][:, 4 * q:4 * q + 4, :])
                    ts.append(t)
                return ts

            # PE warm-up: dummy matmuls keep the HAM activity monitor busy
            # during the initial DMA latency so the first real matmuls run
            # at full clock
            nc.any.memset(dum_t[:], 0)
            wps = psm.tile([128, NT], F32, tag="ps", name="warm")
            for _ in range(8):
                nc.tensor.matmul(wps[:], dum_t[:, 0:128], dum_t[:],
                                 start=True, stop=True, skip_group_check=True)

            # ---- startup DMAs: everything on the sync queue, in strict
            # first-use order, so the critical first chunks aren't racing
            # other loads for the shared DMA engines
            x8_c, xb_c = [], []
            for q in range(3):
                t = x8p.tile([128, 4, NT], FP8, tag=f"x8_{q}", name=f"x8_0_{q}")
                nc.sync.dma_start(out=t[:], in_=x8d[0][:, 4 * q:4 * q + 4, :])
                x8_c.append(t)
                nc.sync.dma_start(out=a8_c[q][:],
                                  in_=a8d[:, 4 * q:4 * q + 4, :])
            nc.sync.dma_start(out=m8_t[:], in_=m8d[:])
            nc.sync.dma_start(out=w_c[0][0][:], in_=wd[0][:, 0:4, :])
            t = xbp.tile([128, 4, NT], BF16, tag="xb_0", name="xb_0_0")
            nc.sync.dma_start(out=t[:], in_=xbd[0][:, 0:4, :])
            xb_c.append(t)
            for q in range(3, NQ):
                t = x8p.tile([128, 4, NT], FP8, tag=f"x8_{q}", name=f"x8_0_{q}")
                nc.sync.dma_start(out=t[:], in_=x8d[0][:, 4 * q:4 * q + 4, :])
                x8_c.append(t)
                nc.sync.dma_start(out=a8_c[q][:],
                                  in_=a8d[:, 4 * q:4 * q + 4, :])
            nc.any.memset(st_all[0][:, 1, :], 0)
            nc.sync.dma_start(out=st_all[0][0:8, 1, :], in_=oh8d[:])
            nc.sync.dma_start(out=bcl_t[:], in_=bcld[:])
            for q in range(1, NQ):
                nc.sync.dma_start(out=w_c[0][q][:],
                                  in_=wd[0][:, 4 * q:4 * q + 4, :])
                t = xbp.tile([128, 4, NT], BF16, tag=f"xb_{q}", name=f"xb_0_{q}")
                nc.sync.dma_start(out=t[:], in_=xbd[0][:, 4 * q:4 * q + 4, :])
                xb_c.append(t)
            nc.sync.dma_start(out=w8_c[0][:], in_=w8d[0])
            for q in range(NQ):
                nc.sync.dma_start(out=w_c[1][q][:],
                                  in_=wd[1][:, 4 * q:4 * q + 4, :])
            nc.sync.dma_start(out=w8_c[1][:], in_=w8d[1])
            nc.sync.dma_start(out=ba_t[:], in_=bad[:])
            for s in (1, 2):
                nc.any.memset(st_all[s][:, 1, :], 0)
                nc.sync.dma_start(out=st_all[s][0:8, 1, :], in_=oh8d[:])
            for q in range(NQ):
                nc.sync.dma_start(out=w_c[2][q][:],
                                  in_=wd[2][:, 4 * q:4 * q + 4, :])
            nc.sync.dma_start(out=w8_c[2][:], in_=w8d[2])

            for tt in range(ntt):
                tsl = slice(tt * NT, (tt + 1) * NT)
                # ---- shrink: fp8 DoubleRow, all 3 slices ----
                for s in range(3):
                    ps = psm.tile([128, NT], F32, tag="ps", name=f"shr{s}_{tt}")
                    for k in range(NKT // 2):
                        nc.tensor.matmul(
                            ps[:],
                            a8_c[k // 2][:, 2 * (k % 2):2 * (k % 2) + 2,
                                         s * 128:(s + 1) * 128],
                            x8_c[k // 2][:, 2 * (k % 2):2 * (k % 2) + 2, :],
                            start=(k == 0), stop=(k == NKT // 2 - 1),
                            perf_mode=DR, skip_group_check=True,
                        )
                    nc.vector.tensor_mul(st_all[s][:, 0, tsl], ps[:], m8_t[:, tsl])

                # prefetch next token tile while mains chew
                if tt + 1 < ntt:
                    x8_next = load_x8(tt + 1)
                    xb_next = load_xb(tt + 1)
                else:
                    x8_next = xb_next = None

                # ---- main: sub-passes of channel blocks. Each bank opens
                # with a base bf16 matmul (start=True), the fp8 DoubleRow
                # lora-expand accumulates behind it, then the remaining
                # base k-tiles drain the bank.
                def xb_i(i):
                    return xb_c[i // 4][:, i % 4, :]

                def run_group(p, jlist, out_dma=nc.gpsimd.dma_start):
                    pss = {}
                    for j in jlist:
                        blk = j - p * WBLK
                        ps = psm.tile([128, NT], F32, tag="ps", name=f"ps{j}_{tt}")
                        pss[j] = ps
                        nc.tensor.matmul(
                            ps[:],
                            w_c[p][0][:, 0, blk * 128:(blk + 1) * 128],
                            xb_i(0),
                            start=True, stop=False, skip_group_check=True,
                        )
                    for j in jlist:
                        s = slice_of(j)
                        nc.tensor.matmul(
                            pss[j][:],
                            bcl_t[:, :, j * 128:(j + 1) * 128],
                            st_all[s][:, :, tsl],
                            start=False, stop=False,
                            perf_mode=DR, skip_group_check=True,
                        )
                    for j in jlist:
                        blk = j - p * WBLK
                        for i in range(1, NKT - 2):
                            nc.tensor.matmul(
                                pss[j][:],
                                w_c[p][i // 4][:, i % 4,
                                               blk * 128:(blk + 1) * 128],
                                xb_i(i),
                                start=False, stop=False,
                                skip_group_check=True,
                            )
                        nc.tensor.matmul(
                            pss[j][:],
                            w8_c[p][:, :, blk * 128:(blk + 1) * 128],
                            x8_c[3][:, 2:4, :],
                            start=False, stop=True,
                            perf_mode=DR, skip_group_check=True,
                        )
                        o_t = op.tile([128, NT], F32, tag="o")
                        nc.vector.tensor_scalar_add(o_t[:], pss[j][:],
                                                    ba_t[:, j:j + 1])
                        out_dma(
                            out=outT[j * 128:(j + 1) * 128, tsl], in_=o_t[:])

                for p in range(NPASS):
                    js = list(range(p * WBLK, (p + 1) * WBLK))
                    if tt == ntt - 1 and p == NPASS - 1:
                        # split the final sub-pass so the last outputs'
                        # DMA drains earlier
                        run_group(p, js[:4])
                        run_group(p, js[4:6], out_dma=nc.sync.dma_start)
                        run_group(p, js[6:], out_dma=nc.sync.dma_start)
                    else:
                        run_group(p, js)
                if x8_next is not None:
                    x8_c, xb_c = x8_next, xb_next
    nc.compile()
    return nc


_nc_cache = {}


def _get_program(tc_tokens=TC):
    if tc_tokens not in _nc_cache:
        _nc_cache[tc_tokens] = build_program(tc_tokens)
    return _nc_cache[tc_tokens]


def make_in_maps(x, W_qkv, bias_qkv, lora_a_q, lora_a_k, lora_a_v,
                 lora_b_q, lora_b_k, lora_b_v,
                 lora_bias_q, lora_bias_k, lora_bias_v,
                 token_lora_indices, ncores=NCORES):
    x = np.asarray(x, np.float32)
    idx = np.asarray(token_lora_indices).astype(np.int64)
    tc_tokens = x.shape[0] // ncores
    ntt = tc_tokens // NT

    # W: (NPASS, 128, NKT, WBLK*128); [pi, p, i, m] = W_qkv[pi*1024+m, i*128+p]
    wd = np.ascontiguousarray(
        np.asarray(W_qkv, np.float32).reshape(NPASS, WBLK * 128, NKT, 128)
        .transpose(0, 3, 2, 1)).astype(BF16NP)
    w8d_arr = np.ascontiguousarray(
        np.asarray(W_qkv, np.float32).reshape(NPASS, WBLK * 128, NKT, 128)
        [:, :, NKT - 2:, :].transpose(0, 3, 2, 1)).astype(E4NP)
    # A8: (128, NKT, 384); [p, i, m] = 8 * A_stack[m, i*128+p]
    a_stack = np.concatenate([
        np.asarray(lora_a_q, np.float32).reshape(L * R, D),
        np.asarray(lora_a_k, np.float32).reshape(L * R, D),
        np.asarray(lora_a_v, np.float32).reshape(L * R, D)], axis=0)
    a8d = np.ascontiguousarray(
        (a_stack * 8.0).reshape(384, NKT, 128).transpose(2, 1, 0)).astype(E4NP)
    # bcl: (128, 2, O); [:,0,:] = 8*B^T rows (l*R+r), [:8,1,:] = 8*lora_bias
    bcomb = np.concatenate([
        np.asarray(lora_b_q, np.float32).transpose(0, 2, 1).reshape(L * R, QS),
        np.asarray(lora_b_k, np.float32).transpose(0, 2, 1).reshape(L * R, KVS),
        np.asarray(lora_b_v, np.float32).transpose(0, 2, 1).reshape(L * R, KVS)],
        axis=1)
    biasL = np.concatenate([
        np.asarray(lora_bias_q, np.float32),
        np.asarray(lora_bias_k, np.float32),
        np.asarray(lora_bias_v, np.float32)], axis=1)
    bcld = np.zeros((128, 2, O), np.float32)
    bcld[:, 0, :] = bcomb * 8.0
    bcld[:8, 1, :] = biasL * 8.0
    bcld = bcld.astype(E4NP)
    bad = np.ascontiguousarray(
        np.asarray(bias_qkv, np.float32).reshape(NBLK, 128).T)
    lane = np.arange(128) // R

    in_maps = []
    for c in range(ncores):
        sl = slice(c * tc_tokens, (c + 1) * tc_tokens)
        xc = x[sl]
        # (ntt, 128, NKT, NT); [tt, p, i, n] = x[tt*NT+n, i*128+p]
        xr = np.ascontiguousarray(
            xc.reshape(ntt, NT, NKT, 128).transpose(0, 3, 2, 1))
        idx_c = idx[sl]
        in_maps.append({
            "x8d": xr.astype(E4NP),
            "xbd": xr.astype(BF16NP),
            "wd": wd,
            "w8d": w8d_arr,
            "a8d": a8d,
            "bcld": bcld,
            "m8d": np.where(idx_c[None, :] == lane[:, None],
                            np.float32(1 / 64), np.float32(0)).astype(E4NP),
            "oh8d": np.where(idx_c[None, :] == np.arange(L)[:, None],
                             np.float32(0.125), np.float32(0)).astype(E4NP),
            "bad": bad,
        })
    return in_maps, tc_tokens


def kernel(x, W_qkv, bias_qkv, lora_a_q, lora_a_k, lora_a_v,
           lora_b_q, lora_b_k, lora_b_v,
           lora_bias_q, lora_bias_k, lora_bias_v,
           token_lora_indices):
    in_maps, tc_tokens = make_in_maps(
        x, W_qkv, bias_qkv, lora_a_q, lora_a_k, lora_a_v,
        lora_b_q, lora_b_k, lora_b_v,
        lora_bias_q, lora_bias_k, lora_bias_v, token_lora_indices)
    nc = _get_program(tc_tokens)
    res = None
    for attempt in range(3):
        try:
            res = run_bass_kernel_spmd(nc, in_maps, list(range(NCORES)))
            break
        except Exception:
            if attempt == 2:
                raise
    out = np.empty((tc_tokens * NCORES, O), np.float32)
    for c in range(NCORES):
        out[c * tc_tokens:(c + 1) * tc_tokens] = res.results[c]["outT"].T
    return out


# revision 23
# speedup vs baseline: 1.0635x; 1.0043x over previous
"""MergedQKVParallelLinearWithLora on 8 TRN2 NeuronCores.

Token-parallel across the 8 cores: each core computes 4096 tokens of the
full (T=32768, O=3072) output. Per core, per 512-token tile:

  shrink:  s~ = (1/64) * mask * (x8 @ (8*A)^T)   fp8e4 DoubleRow, K=2048
  main:    base bf16 matmul k-tile 0 (start=True)
         + lora expand+bias (one fp8 DoubleRow matmul, K=256: subtile0 =
           s~ rows vs 8*B columns, subtile1 = oh/8 rows vs 8*lora_bias
           rows, zero padded) accumulating into the same PSUM bank
         + base bf16 matmuls k-tiles 1..15
         + per-channel bias (DVE tensor_scalar_add at PSUM eviction)

W (bf16, 96KB/partition) stays fully SBUF-resident, so x streams exactly
once per dtype (bf16 for the base matmul, fp8 pair-layout for the shrink).
Tile-0 inputs and W are loaded through separate 4-ktile chunk tiles so the
first consumers wait on one chunk DMA, not the whole tensor. All
reshapes/transposes/dtype casts are host-side; fp8 scales are chosen so
every operand sits in e4m3's normal range (A,B,bias *8; s~ /8; the product
scales cancel).
"""

import numpy as np
import ml_dtypes

import concourse.mybir as mybir
import concourse.tile as tile
from concourse import bacc
from concourse.bass_utils import run_bass_kernel_spmd

T, D, QS, KVS, L, R = 32768, 2048, 2048, 512, 8, 16
O = QS + 2 * KVS          # 3072
NCORES = 8
TC = T // NCORES          # 4096 tokens per core
NT = 512                  # tokens per tile (matmul moving dim)
NTT = TC // NT            # 8 token tiles
NKT = D // 128            # 16 contraction k-tiles
NBLK = O // 128           # 24 output-channel blocks
WBLK = 8                  # blocks per sub-pass (3 sub-passes)
NPASS = NBLK // WBLK
NQ = 4                    # k-chunks per 16-ktile load (4 ktiles each)

F32 = mybir.dt.float32
BF16 = mybir.dt.bfloat16
FP8 = mybir.dt.float8e4
DR = mybir.MatmulPerfMode.DoubleRow
BF16NP = ml_dtypes.bfloat16
E4NP = ml_dtypes.float8_e4m3


def build_program(tc_tokens=TC):
    ntt = tc_tokens // NT
    nc = bacc.Bacc(None, target_bir_lowering=False, debug=False)

    x8d = nc.dram_tensor("x8d", [ntt, 128, NKT, NT], FP8, kind="ExternalInput")
    xbd = nc.dram_tensor("xbd", [ntt, 128, NKT, NT], BF16, kind="ExternalInput")
    wd = nc.dram_tensor("wd", [NPASS, 128, NKT, WBLK * 128], BF16,
                        kind="ExternalInput")
    a8d = nc.dram_tensor("a8d", [128, NKT, 384], FP8, kind="ExternalInput")
    w8d = nc.dram_tensor("w8d", [NPASS, 128, 2, WBLK * 128], FP8,
                         kind="ExternalInput")
    bcld = nc.dram_tensor("bcld", [128, 2, O], FP8, kind="ExternalInput")
    m8d = nc.dram_tensor("m8d", [128, tc_tokens], FP8, kind="ExternalInput")
    oh8d = nc.dram_tensor("oh8d", [8, tc_tokens], FP8, kind="ExternalInput")
    bad = nc.dram_tensor("bad", [128, NBLK], F32, kind="ExternalInput")
    outT = nc.dram_tensor("outT", [O, tc_tokens], F32, kind="ExternalOutput")

    def slice_of(j):
        return 0 if j < QS // 128 else (1 if j < (QS + KVS) // 128 else 2)

    with tile.TileContext(nc) as tc:
        with tc.tile_pool(name="const", bufs=1) as const, \
             tc.tile_pool(name="x8p", bufs=2) as x8p, \
             tc.tile_pool(name="xbp", bufs=2) as xbp, \
             tc.tile_pool(name="psm", bufs=8, space="PSUM") as psm, \
             tc.tile_pool(name="op", bufs=4) as op:
            # A8 and W as separate 4-ktile chunk tiles: consumers then wait
            # on one chunk's DMA instead of the full tensor
            a8_c = [const.tile([128, 4, 384], FP8, tag=f"a8_{q}",
                               name=f"a8_{q}") for q in range(NQ)]
            w_c = [[const.tile([128, 4, WBLK * 128], BF16, tag=f"w{p}_{q}",
                               name=f"w{p}_{q}") for q in range(NQ)]
                   for p in range(NPASS)]
            w8_c = [const.tile([128, 2, WBLK * 128], FP8, tag=f"w8_{p}",
                               name=f"w8_{p}") for p in range(NPASS)]
            dum_t = const.tile([128, NT], BF16, tag="dum")
            bcl_t = const.tile([128, 2, O], FP8, tag="bcl")
            m8_t = const.tile([128, tc_tokens], FP8, tag="m8")
            ba_t = const.tile([128, NBLK], F32, tag="ba")
            st_all = [const.tile([128, 2, tc_tokens], FP8, tag=f"st{s}",
                                 name=f"st{s}") for s in range(3)]
            def load_x8(tt):
                ts = []
                for q in range(NQ):
                    t = x8p.tile([128, 4, NT], FP8, tag=f"x8_{q}",
                                 name=f"x8_{tt}_{q}")
                    nc.sync.dma_start(out=t[:], in_=x8d[tt][:, 4 * q:4 * q + 4, :])
                    ts.append(t)
                return ts

            def load_xb(tt):
                # k-tiles 14,15 ride the fp8 stream instead; chunk 3 only
                # carries 12,13
                ts = []
                for q in range(NQ - 1):
                    t = xbp.tile([128, 4, NT], BF16, tag=f"xb_{q}",
                                 name=f"xb_{tt}_{q}")
                    nc.sync.dma_start(out=t[:], in_=xbd[tt][:, 4 * q:4 * q + 4, :])
                    ts.append(t)
                t = xbp.tile([128, 2, NT], BF16, tag="xb_3", name=f"xb_{tt}_3")
                nc.sync.dma_start(out=t[:], in_=xbd[tt][:, 12:14, :])
                ts.append(t)
                return ts

            # PE warm-up: dummy matmuls keep the HAM activity monitor busy
            # during the initial DMA latency so the first real matmuls run
            # at full clock
            nc.any.memset(dum_t[:], 0)
            wps = psm.tile([128, NT], F32, tag="ps", name="warm")
            for _ in range(8):
                nc.tensor.matmul(wps[:], dum_t[:, 0:128], dum_t[:],
                                 start=True, stop=True, skip_group_check=True)

            # ---- startup DMAs: everything on the sync queue, in strict
            # first-use order, so the critical first chunks aren't racing
            # other loads for the shared DMA engines
            x8_c, xb_c = [], []
            for q in range(3):
                t = x8p.tile([128, 4, NT], FP8, tag=f"x8_{q}", name=f"x8_0_{q}")
                nc.sync.dma_start(out=t[:], in_=x8d[0][:, 4 * q:4 * q + 4, :])
                x8_c.append(t)
                nc.sync.dma_start(out=a8_c[q][:],
                                  in_=a8d[:, 4 * q:4 * q + 4, :])
            nc.sync.dma_start(out=m8_t[:], in_=m8d[:])
            nc.sync.dma_start(out=w_c[0][0][:], in_=wd[0][:, 0:4, :])
            t = xbp.tile([128, 4, NT], BF16, tag="xb_0", name="xb_0_0")
            nc.sync.dma_start(out=t[:], in_=xbd[0][:, 0:4, :])
            xb_c.append(t)
            for q in range(3, NQ):
                t = x8p.tile([128, 4, NT], FP8, tag=f"x8_{q}", name=f"x8_0_{q}")
                nc.sync.dma_start(out=t[:], in_=x8d[0][:, 4 * q:4 * q + 4, :])
                x8_c.append(t)
                nc.sync.dma_start(out=a8_c[q][:],
                                  in_=a8d[:, 4 * q:4 * q + 4, :])
            nc.any.memset(st_all[0][:, 1, :], 0)
            nc.sync.dma_start(out=st_all[0][0:8, 1, :], in_=oh8d[:])
            nc.sync.dma_start(out=bcl_t[:], in_=bcld[:])
            for q in range(1, NQ - 1):
                nc.sync.dma_start(out=w_c[0][q][:],
                                  in_=wd[0][:, 4 * q:4 * q + 4, :])
                t = xbp.tile([128, 4, NT], BF16, tag=f"xb_{q}", name=f"xb_0_{q}")
                nc.sync.dma_start(out=t[:], in_=xbd[0][:, 4 * q:4 * q + 4, :])
                xb_c.append(t)
            nc.sync.dma_start(out=w_c[0][3][:], in_=wd[0][:, 12:16, :])
            t = xbp.tile([128, 2, NT], BF16, tag="xb_3", name="xb_0_3")
            nc.sync.dma_start(out=t[:], in_=xbd[0][:, 12:14, :])
            xb_c.append(t)
            nc.sync.dma_start(out=w8_c[0][:], in_=w8d[0])
            for q in range(NQ):
                nc.sync.dma_start(out=w_c[1][q][:],
                                  in_=wd[1][:, 4 * q:4 * q + 4, :])
            nc.sync.dma_start(out=w8_c[1][:], in_=w8d[1])
            nc.sync.dma_start(out=ba_t[:], in_=bad[:])
            for s in (1, 2):
                nc.any.memset(st_all[s][:, 1, :], 0)
                nc.sync.dma_start(out=st_all[s][0:8, 1, :], in_=oh8d[:])
            for q in range(NQ):
                nc.sync.dma_start(out=w_c[2][q][:],
                                  in_=wd[2][:, 4 * q:4 * q + 4, :])
            nc.sync.dma_start(out=w8_c[2][:], in_=w8d[2])

            for tt in range(ntt):
                tsl = slice(tt * NT, (tt + 1) * NT)
                # ---- shrink: fp8 DoubleRow, all 3 slices ----
                for s in range(3):
                    ps = psm.tile([128, NT], F32, tag="ps", name=f"shr{s}_{tt}")
                    for k in range(NKT // 2):
                        nc.tensor.matmul(
                            ps[:],
                            a8_c[k // 2][:, 2 * (k % 2):2 * (k % 2) + 2,
                                         s * 128:(s + 1) * 128],
                            x8_c[k // 2][:, 2 * (k % 2):2 * (k % 2) + 2, :],
                            start=(k == 0), stop=(k == NKT // 2 - 1),
                            perf_mode=DR, skip_group_check=True,
                        )
                    nc.vector.tensor_mul(st_all[s][:, 0, tsl], ps[:], m8_t[:, tsl])

                # prefetch next token tile while mains chew
                if tt + 1 < ntt:
                    x8_next = load_x8(tt + 1)
                    xb_next = load_xb(tt + 1)
                else:
                    x8_next = xb_next = None

                # ---- main: sub-passes of channel blocks. Each bank opens
                # with a base bf16 matmul (start=True), the fp8 DoubleRow
                # lora-expand accumulates behind it, then the remaining
                # base k-tiles drain the bank.
                def xb_i(i):
                    return xb_c[3][:, i - 12, :] if i >= 12 else \
                        xb_c[i // 4][:, i % 4, :]

                def run_group(p, jlist, out_dma=nc.gpsimd.dma_start):
                    pss = {}
                    for j in jlist:
                        blk = j - p * WBLK
                        ps = psm.tile([128, NT], F32, tag="ps", name=f"ps{j}_{tt}")
                        pss[j] = ps
                        nc.tensor.matmul(
                            ps[:],
                            w_c[p][0][:, 0, blk * 128:(blk + 1) * 128],
                            xb_i(0),
                            start=True, stop=False, skip_group_check=True,
                        )
                    for j in jlist:
                        s = slice_of(j)
                        nc.tensor.matmul(
                            pss[j][:],
                            bcl_t[:, :, j * 128:(j + 1) * 128],
                            st_all[s][:, :, tsl],
                            start=False, stop=False,
                            perf_mode=DR, skip_group_check=True,
                        )
                    for j in jlist:
                        blk = j - p * WBLK
                        for i in range(1, NKT - 2):
                            nc.tensor.matmul(
                                pss[j][:],
                                w_c[p][i // 4][:, i % 4,
                                               blk * 128:(blk + 1) * 128],
                                xb_i(i),
                                start=False, stop=False,
                                skip_group_check=True,
                            )
                        nc.tensor.matmul(
                            pss[j][:],
                            w8_c[p][:, :, blk * 128:(blk + 1) * 128],
                            x8_c[3][:, 2:4, :],
                            start=False, stop=True,
                            perf_mode=DR, skip_group_check=True,
                        )
                        o_t = op.tile([128, NT], F32, tag="o")
                        nc.vector.tensor_scalar_add(o_t[:], pss[j][:],
                                                    ba_t[:, j:j + 1])
                        out_dma(
                            out=outT[j * 128:(j + 1) * 128, tsl], in_=o_t[:])

                for p in range(NPASS):
                    js = list(range(p * WBLK, (p + 1) * WBLK))
                    if tt == ntt - 1 and p == NPASS - 1:
                        # split the final sub-pass so the last outputs'
                        # DMA drains earlier
                        run_group(p, js[:4])
                        run_group(p, js[4:6], out_dma=nc.sync.dma_start)
                        run_group(p, js[6:], out_dma=nc.sync.dma_start)
                    else:
                        run_group(p, js)
                if x8_next is not None:
                    x8_c, xb_c = x8_next, xb_next
    nc.compile()
    return nc


_nc_cache = {}


def _get_program(tc_tokens=TC):
    if tc_tokens not in _nc_cache:
        _nc_cache[tc_tokens] = build_program(tc_tokens)
    return _nc_cache[tc_tokens]


def make_in_maps(x, W_qkv, bias_qkv, lora_a_q, lora_a_k, lora_a_v,
                 lora_b_q, lora_b_k, lora_b_v,
                 lora_bias_q, lora_bias_k, lora_bias_v,
                 token_lora_indices, ncores=NCORES):
    x = np.asarray(x, np.float32)
    idx = np.asarray(token_lora_indices).astype(np.int64)
    tc_tokens = x.shape[0] // ncores
    ntt = tc_tokens // NT

    # W: (NPASS, 128, NKT, WBLK*128); [pi, p, i, m] = W_qkv[pi*1024+m, i*128+p]
    wd = np.ascontiguousarray(
        np.asarray(W_qkv, np.float32).reshape(NPASS, WBLK * 128, NKT, 128)
        .transpose(0, 3, 2, 1)).astype(BF16NP)
    w8d_arr = np.ascontiguousarray(
        np.asarray(W_qkv, np.float32).reshape(NPASS, WBLK * 128, NKT, 128)
        [:, :, NKT - 2:, :].transpose(0, 3, 2, 1)).astype(E4NP)
    # A8: (128, NKT, 384); [p, i, m] = 8 * A_stack[m, i*128+p]
    a_stack = np.concatenate([
        np.asarray(lora_a_q, np.float32).reshape(L * R, D),
        np.asarray(lora_a_k, np.float32).reshape(L * R, D),
        np.asarray(lora_a_v, np.float32).reshape(L * R, D)], axis=0)
    a8d = np.ascontiguousarray(
        (a_stack * 8.0).reshape(384, NKT, 128).transpose(2, 1, 0)).astype(E4NP)
    # bcl: (128, 2, O); [:,0,:] = 8*B^T rows (l*R+r), [:8,1,:] = 8*lora_bias
    bcomb = np.concatenate([
        np.asarray(lora_b_q, np.float32).transpose(0, 2, 1).reshape(L * R, QS),
        np.asarray(lora_b_k, np.float32).transpose(0, 2, 1).reshape(L * R, KVS),
        np.asarray(lora_b_v, np.float32).transpose(0, 2, 1).reshape(L * R, KVS)],
        axis=1)
    biasL = np.concatenate([
        np.asarray(lora_bias_q, np.float32),
        np.asarray(lora_bias_k, np.float32),
        np.asarray(lora_bias_v, np.float32)], axis=1)
    bcld = np.zeros((128, 2, O), np.float32)
    bcld[:, 0, :] = bcomb * 8.0
    bcld[:8, 1, :] = biasL * 8.0
    bcld = bcld.astype(E4NP)
    bad = np.ascontiguousarray(
        np.asarray(bias_qkv, np.float32).reshape(NBLK, 128).T)
    lane = np.arange(128) // R

    in_maps = []
    for c in range(ncores):
        sl = slice(c * tc_tokens, (c + 1) * tc_tokens)
        xc = x[sl]
        # (ntt, 128, NKT, NT); [tt, p, i, n] = x[tt*NT+n, i*128+p]
        xr = np.ascontiguousarray(
            xc.reshape(ntt, NT, NKT, 128).transpose(0, 3, 2, 1))
        idx_c = idx[sl]
        in_maps.append({
            "x8d": xr.astype(E4NP),
            "xbd": xr.astype(BF16NP),
            "wd": wd,
            "w8d": w8d_arr,
            "a8d": a8d,
            "bcld": bcld,
            "m8d": np.where(idx_c[None, :] == lane[:, None],
                            np.float32(1 / 64), np.float32(0)).astype(E4NP),
            "oh8d": np.where(idx_c[None, :] == np.arange(L)[:, None],
                             np.float32(0.125), np.float32(0)).astype(E4NP),
            "bad": bad,
        })
    return in_maps, tc_tokens


def kernel(x, W_qkv, bias_qkv, lora_a_q, lora_a_k, lora_a_v,
           lora_b_q, lora_b_k, lora_b_v,
           lora_bias_q, lora_bias_k, lora_bias_v,
           token_lora_indices):
    in_maps, tc_tokens = make_in_maps(
        x, W_qkv, bias_qkv, lora_a_q, lora_a_k, lora_a_v,
        lora_b_q, lora_b_k, lora_b_v,
        lora_bias_q, lora_bias_k, lora_bias_v, token_lora_indices)
    nc = _get_program(tc_tokens)
    res = None
    for attempt in range(3):
        try:
            res = run_bass_kernel_spmd(nc, in_maps, list(range(NCORES)))
            break
        except Exception:
            if attempt == 2:
                raise
    out = np.empty((tc_tokens * NCORES, O), np.float32)
    for c in range(NCORES):
        out[c * tc_tokens:(c + 1) * tc_tokens] = res.results[c]["outT"].T
    return out


# revision 24
# speedup vs baseline: 1.0679x; 1.0041x over previous
"""MergedQKVParallelLinearWithLora on 8 TRN2 NeuronCores.

Token-parallel across the 8 cores: each core computes 4096 tokens of the
full (T=32768, O=3072) output. Per core, per 512-token tile:

  shrink:  s~ = (1/64) * mask * (x8 @ (8*A)^T)   fp8e4 DoubleRow, K=2048
  main:    base bf16 matmul k-tile 0 (start=True)
         + lora expand+bias (one fp8 DoubleRow matmul, K=256: subtile0 =
           s~ rows vs 8*B columns, subtile1 = oh/8 rows vs 8*lora_bias
           rows, zero padded) accumulating into the same PSUM bank
         + base bf16 matmuls k-tiles 1..15
         + per-channel bias (DVE tensor_scalar_add at PSUM eviction)

W (bf16, 96KB/partition) stays fully SBUF-resident, so x streams exactly
once per dtype (bf16 for the base matmul, fp8 pair-layout for the shrink).
Tile-0 inputs and W are loaded through separate 4-ktile chunk tiles so the
first consumers wait on one chunk DMA, not the whole tensor. All
reshapes/transposes/dtype casts are host-side; fp8 scales are chosen so
every operand sits in e4m3's normal range (A,B,bias *8; s~ /8; the product
scales cancel).
"""

import numpy as np
import ml_dtypes

import concourse.mybir as mybir
import concourse.tile as tile
from concourse import bacc
from concourse.bass_utils import run_bass_kernel_spmd

T, D, QS, KVS, L, R = 32768, 2048, 2048, 512, 8, 16
O = QS + 2 * KVS          # 3072
NCORES = 8
TC = T // NCORES          # 4096 tokens per core
NT = 512                  # tokens per tile (matmul moving dim)
NTT = TC // NT            # 8 token tiles
NKT = D // 128            # 16 contraction k-tiles
NBLK = O // 128           # 24 output-channel blocks
WBLK = 8                  # blocks per sub-pass (3 sub-passes)
NPASS = NBLK // WBLK
NQ = 4                    # k-chunks per 16-ktile load (4 ktiles each)

F32 = mybir.dt.float32
BF16 = mybir.dt.bfloat16
FP8 = mybir.dt.float8e4
DR = mybir.MatmulPerfMode.DoubleRow
BF16NP = ml_dtypes.bfloat16
E4NP = ml_dtypes.float8_e4m3


def build_program(tc_tokens=TC):
    ntt = tc_tokens // NT
    nc = bacc.Bacc(None, target_bir_lowering=False, debug=False)

    x8d = nc.dram_tensor("x8d", [ntt, 128, NKT, NT], FP8, kind="ExternalInput")
    xbd = nc.dram_tensor("xbd", [ntt, 128, NKT, NT], BF16, kind="ExternalInput")
    wd = nc.dram_tensor("wd", [NPASS, 128, NKT, WBLK * 128], BF16,
                        kind="ExternalInput")
    a8d = nc.dram_tensor("a8d", [128, NKT, 384], FP8, kind="ExternalInput")
    w8d = nc.dram_tensor("w8d", [NPASS, 128, 2, WBLK * 128], FP8,
                         kind="ExternalInput")
    bcld = nc.dram_tensor("bcld", [128, 2, O], FP8, kind="ExternalInput")
    m8d = nc.dram_tensor("m8d", [128, tc_tokens], FP8, kind="ExternalInput")
    oh8d = nc.dram_tensor("oh8d", [8, tc_tokens], FP8, kind="ExternalInput")
    bad = nc.dram_tensor("bad", [128, NBLK], F32, kind="ExternalInput")
    outT = nc.dram_tensor("outT", [O, tc_tokens], F32, kind="ExternalOutput")

    def slice_of(j):
        return 0 if j < QS // 128 else (1 if j < (QS + KVS) // 128 else 2)

    with tile.TileContext(nc) as tc:
        with tc.tile_pool(name="const", bufs=1) as const, \
             tc.tile_pool(name="x8p", bufs=2) as x8p, \
             tc.tile_pool(name="xbp", bufs=2) as xbp, \
             tc.tile_pool(name="psm", bufs=8, space="PSUM") as psm, \
             tc.tile_pool(name="op", bufs=4) as op:
            # A8 and W as separate 4-ktile chunk tiles: consumers then wait
            # on one chunk's DMA instead of the full tensor
            a8_c = [const.tile([128, 4, 384], FP8, tag=f"a8_{q}",
                               name=f"a8_{q}") for q in range(NQ)]
            w_c = [[const.tile([128, 4 if q < 3 else 2, WBLK * 128], BF16,
                               tag=f"w{p}_{q}", name=f"w{p}_{q}")
                    for q in range(NQ)] for p in range(NPASS)]
            w8_c = [const.tile([128, 2, WBLK * 128], FP8, tag=f"w8_{p}",
                               name=f"w8_{p}") for p in range(NPASS)]
            dum_t = const.tile([128, NT], BF16, tag="dum")
            bcl_t = const.tile([128, 2, O], FP8, tag="bcl")
            m8_t = const.tile([128, tc_tokens], FP8, tag="m8")
            ba_t = const.tile([128, NBLK], F32, tag="ba")
            st_all = [const.tile([128, 2, tc_tokens], FP8, tag=f"st{s}",
                                 name=f"st{s}") for s in range(3)]
            def load_x8(tt):
                ts = []
                for q in range(NQ):
                    t = x8p.tile([128, 4, NT], FP8, tag=f"x8_{q}",
                                 name=f"x8_{tt}_{q}")
                    nc.sync.dma_start(out=t[:], in_=x8d[tt][:, 4 * q:4 * q + 4, :])
                    ts.append(t)
                return ts

            def load_xb(tt):
                # k-tiles 14,15 ride the fp8 stream instead; chunk 3 only
                # carries 12,13
                ts = []
                for q in range(NQ - 1):
                    t = xbp.tile([128, 4, NT], BF16, tag=f"xb_{q}",
                                 name=f"xb_{tt}_{q}")
                    nc.sync.dma_start(out=t[:], in_=xbd[tt][:, 4 * q:4 * q + 4, :])
                    ts.append(t)
                t = xbp.tile([128, 2, NT], BF16, tag="xb_3", name=f"xb_{tt}_3")
                nc.sync.dma_start(out=t[:], in_=xbd[tt][:, 12:14, :])
                ts.append(t)
                return ts

            # PE warm-up: dummy matmuls keep the HAM activity monitor busy
            # during the initial DMA latency so the first real matmuls run
            # at full clock
            nc.any.memset(dum_t[:], 0)
            wps = psm.tile([128, NT], F32, tag="ps", name="warm")
            for _ in range(8):
                nc.tensor.matmul(wps[:], dum_t[:, 0:128], dum_t[:],
                                 start=True, stop=True, skip_group_check=True)

            # ---- startup DMAs: everything on the sync queue, in strict
            # first-use order, so the critical first chunks aren't racing
            # other loads for the shared DMA engines
            def load_w(p):
                for q in range(3):
                    nc.sync.dma_start(out=w_c[p][q][:],
                                      in_=wd[p][:, 4 * q:4 * q + 4, :])
                nc.sync.dma_start(out=w_c[p][3][:], in_=wd[p][:, 12:14, :])
                nc.sync.dma_start(out=w8_c[p][:], in_=w8d[p])

            x8_c, xb_c = [], []
            for q in range(NQ):
                t = x8p.tile([128, 4, NT], FP8, tag=f"x8_{q}", name=f"x8_0_{q}")
                nc.sync.dma_start(out=t[:], in_=x8d[0][:, 4 * q:4 * q + 4, :])
                x8_c.append(t)
                nc.sync.dma_start(out=a8_c[q][:],
                                  in_=a8d[:, 4 * q:4 * q + 4, :])
            nc.sync.dma_start(out=m8_t[:], in_=m8d[:])
            nc.sync.dma_start(out=w_c[0][0][:], in_=wd[0][:, 0:4, :])
            t = xbp.tile([128, 4, NT], BF16, tag="xb_0", name="xb_0_0")
            nc.sync.dma_start(out=t[:], in_=xbd[0][:, 0:4, :])
            xb_c.append(t)
            nc.any.memset(st_all[0][:, 1, :], 0)
            nc.sync.dma_start(out=st_all[0][0:8, 1, :], in_=oh8d[:])
            nc.sync.dma_start(out=bcl_t[:], in_=bcld[:])
            for q in range(1, NQ - 1):
                nc.sync.dma_start(out=w_c[0][q][:],
                                  in_=wd[0][:, 4 * q:4 * q + 4, :])
                t = xbp.tile([128, 4, NT], BF16, tag=f"xb_{q}", name=f"xb_0_{q}")
                nc.sync.dma_start(out=t[:], in_=xbd[0][:, 4 * q:4 * q + 4, :])
                xb_c.append(t)
            nc.sync.dma_start(out=w_c[0][3][:], in_=wd[0][:, 12:14, :])
            t = xbp.tile([128, 2, NT], BF16, tag="xb_3", name="xb_0_3")
            nc.sync.dma_start(out=t[:], in_=xbd[0][:, 12:14, :])
            xb_c.append(t)
            nc.sync.dma_start(out=w8_c[0][:], in_=w8d[0])
            load_w(1)
            nc.sync.dma_start(out=ba_t[:], in_=bad[:])
            for s in (1, 2):
                nc.any.memset(st_all[s][:, 1, :], 0)
                nc.sync.dma_start(out=st_all[s][0:8, 1, :], in_=oh8d[:])
            load_w(2)

            for tt in range(ntt):
                tsl = slice(tt * NT, (tt + 1) * NT)
                # ---- shrink: fp8 DoubleRow, all 3 slices ----
                for s in range(3):
                    ps = psm.tile([128, NT], F32, tag="ps", name=f"shr{s}_{tt}")
                    for k in range(NKT // 2):
                        nc.tensor.matmul(
                            ps[:],
                            a8_c[k // 2][:, 2 * (k % 2):2 * (k % 2) + 2,
                                         s * 128:(s + 1) * 128],
                            x8_c[k // 2][:, 2 * (k % 2):2 * (k % 2) + 2, :],
                            start=(k == 0), stop=(k == NKT // 2 - 1),
                            perf_mode=DR, skip_group_check=True,
                        )
                    nc.vector.tensor_mul(st_all[s][:, 0, tsl], ps[:], m8_t[:, tsl])

                # prefetch next token tile while mains chew
                if tt + 1 < ntt:
                    x8_next = load_x8(tt + 1)
                    xb_next = load_xb(tt + 1)
                else:
                    x8_next = xb_next = None

                # ---- main: sub-passes of channel blocks. Each bank opens
                # with a base bf16 matmul (start=True), the fp8 DoubleRow
                # lora-expand accumulates behind it, then the remaining
                # base k-tiles drain the bank.
                def xb_i(i):
                    return xb_c[3][:, i - 12, :] if i >= 12 else \
                        xb_c[i // 4][:, i % 4, :]

                def run_group(p, jlist, out_dma=nc.gpsimd.dma_start):
                    pss = {}
                    for j in jlist:
                        blk = j - p * WBLK
                        ps = psm.tile([128, NT], F32, tag="ps", name=f"ps{j}_{tt}")
                        pss[j] = ps
                        nc.tensor.matmul(
                            ps[:],
                            w_c[p][0][:, 0, blk * 128:(blk + 1) * 128],
                            xb_i(0),
                            start=True, stop=False, skip_group_check=True,
                        )
                    for j in jlist:
                        s = slice_of(j)
                        nc.tensor.matmul(
                            pss[j][:],
                            bcl_t[:, :, j * 128:(j + 1) * 128],
                            st_all[s][:, :, tsl],
                            start=False, stop=False,
                            perf_mode=DR, skip_group_check=True,
                        )
                    for j in jlist:
                        blk = j - p * WBLK
                        for i in range(1, NKT - 2):
                            wsl = (w_c[p][3][:, i - 12, blk * 128:(blk + 1) * 128]
                                   if i >= 12 else
                                   w_c[p][i // 4][:, i % 4,
                                                  blk * 128:(blk + 1) * 128])
                            nc.tensor.matmul(
                                pss[j][:], wsl, xb_i(i),
                                start=False, stop=False,
                                skip_group_check=True,
                            )
                        nc.tensor.matmul(
                            pss[j][:],
                            w8_c[p][:, :, blk * 128:(blk + 1) * 128],
                            x8_c[3][:, 2:4, :],
                            start=False, stop=True,
                            perf_mode=DR, skip_group_check=True,
                        )
                        o_t = op.tile([128, NT], F32, tag="o")
                        nc.vector.tensor_scalar_add(o_t[:], pss[j][:],
                                                    ba_t[:, j:j + 1])
                        out_dma(
                            out=outT[j * 128:(j + 1) * 128, tsl], in_=o_t[:])

                for p in range(NPASS):
                    js = list(range(p * WBLK, (p + 1) * WBLK))
                    if tt == ntt - 1 and p == NPASS - 1:
                        # split the final sub-pass so the last outputs'
                        # DMA drains earlier
                        run_group(p, js[:4])
                        run_group(p, js[4:6], out_dma=nc.sync.dma_start)
                        run_group(p, js[6:], out_dma=nc.sync.dma_start)
                    else:
                        run_group(p, js)
                if x8_next is not None:
                    x8_c, xb_c = x8_next, xb_next
    nc.compile()
    return nc


_nc_cache = {}


def _get_program(tc_tokens=TC):
    if tc_tokens not in _nc_cache:
        _nc_cache[tc_tokens] = build_program(tc_tokens)
    return _nc_cache[tc_tokens]


def make_in_maps(x, W_qkv, bias_qkv, lora_a_q, lora_a_k, lora_a_v,
                 lora_b_q, lora_b_k, lora_b_v,
                 lora_bias_q, lora_bias_k, lora_bias_v,
                 token_lora_indices, ncores=NCORES):
    x = np.asarray(x, np.float32)
    idx = np.asarray(token_lora_indices).astype(np.int64)
    tc_tokens = x.shape[0] // ncores
    ntt = tc_tokens // NT

    # W: (NPASS, 128, NKT, WBLK*128); [pi, p, i, m] = W_qkv[pi*1024+m, i*128+p]
    wd = np.ascontiguousarray(
        np.asarray(W_qkv, np.float32).reshape(NPASS, WBLK * 128, NKT, 128)
        .transpose(0, 3, 2, 1)).astype(BF16NP)
    w8d_arr = np.ascontiguousarray(
        np.asarray(W_qkv, np.float32).reshape(NPASS, WBLK * 128, NKT, 128)
        [:, :, NKT - 2:, :].transpose(0, 3, 2, 1)).astype(E4NP)
    # A8: (128, NKT, 384); [p, i, m] = 8 * A_stack[m, i*128+p]
    a_stack = np.concatenate([
        np.asarray(lora_a_q, np.float32).reshape(L * R, D),
        np.asarray(lora_a_k, np.float32).reshape(L * R, D),
        np.asarray(lora_a_v, np.float32).reshape(L * R, D)], axis=0)
    a8d = np.ascontiguousarray(
        (a_stack * 8.0).reshape(384, NKT, 128).transpose(2, 1, 0)).astype(E4NP)
    # bcl: (128, 2, O); [:,0,:] = 8*B^T rows (l*R+r), [:8,1,:] = 8*lora_bias
    bcomb = np.concatenate([
        np.asarray(lora_b_q, np.float32).transpose(0, 2, 1).reshape(L * R, QS),
        np.asarray(lora_b_k, np.float32).transpose(0, 2, 1).reshape(L * R, KVS),
        np.asarray(lora_b_v, np.float32).transpose(0, 2, 1).reshape(L * R, KVS)],
        axis=1)
    biasL = np.concatenate([
        np.asarray(lora_bias_q, np.float32),
        np.asarray(lora_bias_k, np.float32),
        np.asarray(lora_bias_v, np.float32)], axis=1)
    bcld = np.zeros((128, 2, O), np.float32)
    bcld[:, 0, :] = bcomb * 8.0
    bcld[:8, 1, :] = biasL * 8.0
    bcld = bcld.astype(E4NP)
    bad = np.ascontiguousarray(
        np.asarray(bias_qkv, np.float32).reshape(NBLK, 128).T)
    lane = np.arange(128) // R

    in_maps = []
    for c in range(ncores):
        sl = slice(c * tc_tokens, (c + 1) * tc_tokens)
        xc = x[sl]
        # (ntt, 128, NKT, NT); [tt, p, i, n] = x[tt*NT+n, i*128+p]
        xr = np.ascontiguousarray(
            xc.reshape(ntt, NT, NKT, 128).transpose(0, 3, 2, 1))
        idx_c = idx[sl]
        in_maps.append({
            "x8d": xr.astype(E4NP),
            "xbd": xr.astype(BF16NP),
            "wd": wd,
            "w8d": w8d_arr,
            "a8d": a8d,
            "bcld": bcld,
            "m8d": np.where(idx_c[None, :] == lane[:, None],
                            np.float32(1 / 64), np.float32(0)).astype(E4NP),
            "oh8d": np.where(idx_c[None, :] == np.arange(L)[:, None],
                             np.float32(0.125), np.float32(0)).astype(E4NP),
            "bad": bad,
        })
    return in_maps, tc_tokens


def kernel(x, W_qkv, bias_qkv, lora_a_q, lora_a_k, lora_a_v,
           lora_b_q, lora_b_k, lora_b_v,
           lora_bias_q, lora_bias_k, lora_bias_v,
           token_lora_indices):
    in_maps, tc_tokens = make_in_maps(
        x, W_qkv, bias_qkv, lora_a_q, lora_a_k, lora_a_v,
        lora_b_q, lora_b_k, lora_b_v,
        lora_bias_q, lora_bias_k, lora_bias_v, token_lora_indices)
    nc = _get_program(tc_tokens)
    res = None
    for attempt in range(3):
        try:
            res = run_bass_kernel_spmd(nc, in_maps, list(range(NCORES)))
            break
        except Exception:
            if attempt == 2:
                raise
    out = np.empty((tc_tokens * NCORES, O), np.float32)
    for c in range(NCORES):
        out[c * tc_tokens:(c + 1) * tc_tokens] = res.results[c]["outT"].T
    return out


# revision 25
# speedup vs baseline: 1.0693x; 1.0013x over previous
"""MergedQKVParallelLinearWithLora on 8 TRN2 NeuronCores.

Token-parallel across the 8 cores: each core computes 4096 tokens of the
full (T=32768, O=3072) output. Per core, per 512-token tile:

  shrink:  s~ = (1/64) * mask * (x8 @ (8*A)^T)   fp8e4 DoubleRow, K=2048
  main:    base bf16 matmul k-tile 0 (start=True)
         + lora expand+bias (one fp8 DoubleRow matmul, K=256: subtile0 =
           s~ rows vs 8*B columns, subtile1 = oh/8 rows vs 8*lora_bias
           rows, zero padded) accumulating into the same PSUM bank
         + base bf16 matmuls k-tiles 1..15
         + per-channel bias (DVE tensor_scalar_add at PSUM eviction)

W (bf16, 96KB/partition) stays fully SBUF-resident, so x streams exactly
once per dtype (bf16 for the base matmul, fp8 pair-layout for the shrink).
Tile-0 inputs and W are loaded through separate 4-ktile chunk tiles so the
first consumers wait on one chunk DMA, not the whole tensor. All
reshapes/transposes/dtype casts are host-side; fp8 scales are chosen so
every operand sits in e4m3's normal range (A,B,bias *8; s~ /8; the product
scales cancel).
"""

import numpy as np
import ml_dtypes

import concourse.mybir as mybir
import concourse.tile as tile
from concourse import bacc
from concourse.bass_utils import run_bass_kernel_spmd

T, D, QS, KVS, L, R = 32768, 2048, 2048, 512, 8, 16
O = QS + 2 * KVS          # 3072
NCORES = 8
TC = T // NCORES          # 4096 tokens per core
NT = 512                  # tokens per tile (matmul moving dim)
NTT = TC // NT            # 8 token tiles
NKT = D // 128            # 16 contraction k-tiles
NBLK = O // 128           # 24 output-channel blocks
WBLK = 8                  # blocks per sub-pass (3 sub-passes)
NPASS = NBLK // WBLK
NQ = 4                    # k-chunks per 16-ktile load (4 ktiles each)

F32 = mybir.dt.float32
BF16 = mybir.dt.bfloat16
FP8 = mybir.dt.float8e4
DR = mybir.MatmulPerfMode.DoubleRow
BF16NP = ml_dtypes.bfloat16
E4NP = ml_dtypes.float8_e4m3


def build_program(tc_tokens=TC):
    ntt = tc_tokens // NT
    nc = bacc.Bacc(None, target_bir_lowering=False, debug=False)

    x8d = nc.dram_tensor("x8d", [ntt, 128, NKT, NT], FP8, kind="ExternalInput")
    xbd = nc.dram_tensor("xbd", [ntt, 128, NKT, NT], BF16, kind="ExternalInput")
    wd = nc.dram_tensor("wd", [NPASS, 128, NKT, WBLK * 128], BF16,
                        kind="ExternalInput")
    a8d = nc.dram_tensor("a8d", [128, NKT, 384], FP8, kind="ExternalInput")
    w8d = nc.dram_tensor("w8d", [NPASS, 128, 2, WBLK * 128], FP8,
                         kind="ExternalInput")
    bcld = nc.dram_tensor("bcld", [128, 2, O], FP8, kind="ExternalInput")
    m8d = nc.dram_tensor("m8d", [128, tc_tokens], FP8, kind="ExternalInput")
    oh8d = nc.dram_tensor("oh8d", [8, tc_tokens], FP8, kind="ExternalInput")
    bad = nc.dram_tensor("bad", [128, NBLK], F32, kind="ExternalInput")
    outT = nc.dram_tensor("outT", [O, tc_tokens], F32, kind="ExternalOutput")

    def slice_of(j):
        return 0 if j < QS // 128 else (1 if j < (QS + KVS) // 128 else 2)

    with tile.TileContext(nc) as tc:
        with tc.tile_pool(name="const", bufs=1) as const, \
             tc.tile_pool(name="x8p", bufs=2) as x8p, \
             tc.tile_pool(name="xbp", bufs=2) as xbp, \
             tc.tile_pool(name="psm", bufs=8, space="PSUM") as psm, \
             tc.tile_pool(name="op", bufs=4) as op:
            # A8 and W as separate 4-ktile chunk tiles: consumers then wait
            # on one chunk's DMA instead of the full tensor
            a8_c = [const.tile([128, 4, 384], FP8, tag=f"a8_{q}",
                               name=f"a8_{q}") for q in range(NQ)]
            w_c = [[const.tile([128, 4 if q < 3 else 2, WBLK * 128], BF16,
                               tag=f"w{p}_{q}", name=f"w{p}_{q}")
                    for q in range(NQ)] for p in range(NPASS)]
            w8_c = [const.tile([128, 2, WBLK * 128], FP8, tag=f"w8_{p}",
                               name=f"w8_{p}") for p in range(NPASS)]
            dum_t = const.tile([128, NT], BF16, tag="dum")
            bcl_t = const.tile([128, 2, O], FP8, tag="bcl")
            m8_t = const.tile([128, tc_tokens], FP8, tag="m8")
            ba_t = const.tile([128, NBLK], F32, tag="ba")
            st_all = [const.tile([128, 2, tc_tokens], FP8, tag=f"st{s}",
                                 name=f"st{s}") for s in range(3)]
            def load_x8(tt):
                ts = []
                for q in range(NQ):
                    t = x8p.tile([128, 4, NT], FP8, tag=f"x8_{q}",
                                 name=f"x8_{tt}_{q}")
                    nc.sync.dma_start(out=t[:], in_=x8d[tt][:, 4 * q:4 * q + 4, :])
                    ts.append(t)
                return ts

            def load_xb(tt):
                # k-tiles 14,15 ride the fp8 stream instead; chunk 3 only
                # carries 12,13
                ts = []
                for q in range(NQ - 1):
                    t = xbp.tile([128, 4, NT], BF16, tag=f"xb_{q}",
                                 name=f"xb_{tt}_{q}")
                    nc.sync.dma_start(out=t[:], in_=xbd[tt][:, 4 * q:4 * q + 4, :])
                    ts.append(t)
                t = xbp.tile([128, 2, NT], BF16, tag="xb_3", name=f"xb_{tt}_3")
                nc.sync.dma_start(out=t[:], in_=xbd[tt][:, 12:14, :])
                ts.append(t)
                return ts

            # PE warm-up: dummy matmuls keep the HAM activity monitor busy
            # during the initial DMA latency so the first real matmuls run
            # at full clock
            nc.any.memset(dum_t[:], 0)
            wps = psm.tile([128, NT], F32, tag="ps", name="warm")
            for _ in range(8):
                nc.tensor.matmul(wps[:], dum_t[:, 0:128], dum_t[:],
                                 start=True, stop=True, skip_group_check=True)

            # ---- startup DMAs: everything on the sync queue, in strict
            # first-use order, so the critical first chunks aren't racing
            # other loads for the shared DMA engines
            def load_w(p):
                for q in range(3):
                    nc.sync.dma_start(out=w_c[p][q][:],
                                      in_=wd[p][:, 4 * q:4 * q + 4, :])
                nc.sync.dma_start(out=w_c[p][3][:], in_=wd[p][:, 12:14, :])
                nc.sync.dma_start(out=w8_c[p][:], in_=w8d[p])

            x8_c, xb_c = [], []
            for q in range(NQ):
                t = x8p.tile([128, 4, NT], FP8, tag=f"x8_{q}", name=f"x8_0_{q}")
                nc.sync.dma_start(out=t[:], in_=x8d[0][:, 4 * q:4 * q + 4, :])
                x8_c.append(t)
                nc.sync.dma_start(out=a8_c[q][:],
                                  in_=a8d[:, 4 * q:4 * q + 4, :])
            nc.sync.dma_start(out=m8_t[:], in_=m8d[:])
            nc.sync.dma_start(out=w_c[0][0][:], in_=wd[0][:, 0:4, :])
            t = xbp.tile([128, 4, NT], BF16, tag="xb_0", name="xb_0_0")
            nc.sync.dma_start(out=t[:], in_=xbd[0][:, 0:4, :])
            xb_c.append(t)
            nc.any.memset(st_all[0][:, 1, :], 0)
            nc.sync.dma_start(out=st_all[0][0:8, 1, :], in_=oh8d[:])
            nc.sync.dma_start(out=bcl_t[:], in_=bcld[:])
            for q in range(1, NQ - 1):
                nc.sync.dma_start(out=w_c[0][q][:],
                                  in_=wd[0][:, 4 * q:4 * q + 4, :])
                t = xbp.tile([128, 4, NT], BF16, tag=f"xb_{q}", name=f"xb_0_{q}")
                nc.sync.dma_start(out=t[:], in_=xbd[0][:, 4 * q:4 * q + 4, :])
                xb_c.append(t)
            nc.sync.dma_start(out=w_c[0][3][:], in_=wd[0][:, 12:14, :])
            t = xbp.tile([128, 2, NT], BF16, tag="xb_3", name="xb_0_3")
            nc.sync.dma_start(out=t[:], in_=xbd[0][:, 12:14, :])
            xb_c.append(t)
            nc.sync.dma_start(out=w8_c[0][:], in_=w8d[0])
            load_w(1)
            nc.sync.dma_start(out=ba_t[:], in_=bad[:])
            for s in (1, 2):
                nc.any.memset(st_all[s][:, 1, :], 0)
                nc.sync.dma_start(out=st_all[s][0:8, 1, :], in_=oh8d[:])
            load_w(2)

            for tt in range(ntt):
                tsl = slice(tt * NT, (tt + 1) * NT)
                # ---- shrink: fp8 DoubleRow, all 3 slices. k-outer so the
                # three banks drain each x8 chunk as it lands (3 matmuls of
                # cover per chunk instead of 1 during the startup DMA ramp)
                shr_ps = [psm.tile([128, NT], F32, tag="ps", name=f"shr{s}_{tt}")
                          for s in range(3)]
                for k in range(NKT // 2):
                    for s in range(3):
                        nc.tensor.matmul(
                            shr_ps[s][:],
                            a8_c[k // 2][:, 2 * (k % 2):2 * (k % 2) + 2,
                                         s * 128:(s + 1) * 128],
                            x8_c[k // 2][:, 2 * (k % 2):2 * (k % 2) + 2, :],
                            start=(k == 0), stop=(k == NKT // 2 - 1),
                            perf_mode=DR, skip_group_check=True,
                        )
                for s in range(3):
                    nc.vector.tensor_mul(st_all[s][:, 0, tsl], shr_ps[s][:],
                                         m8_t[:, tsl])

                # prefetch next token tile while mains chew
                if tt + 1 < ntt:
                    x8_next = load_x8(tt + 1)
                    xb_next = load_xb(tt + 1)
                else:
                    x8_next = xb_next = None

                # ---- main: sub-passes of channel blocks. Each bank opens
                # with a base bf16 matmul (start=True), the fp8 DoubleRow
                # lora-expand accumulates behind it, then the remaining
                # base k-tiles drain the bank.
                def xb_i(i):
                    return xb_c[3][:, i - 12, :] if i >= 12 else \
                        xb_c[i // 4][:, i % 4, :]

                def run_group(p, jlist, out_dma=nc.gpsimd.dma_start):
                    pss = {}
                    for j in jlist:
                        blk = j - p * WBLK
                        ps = psm.tile([128, NT], F32, tag="ps", name=f"ps{j}_{tt}")
                        pss[j] = ps
                        nc.tensor.matmul(
                            ps[:],
                            w_c[p][0][:, 0, blk * 128:(blk + 1) * 128],
                            xb_i(0),
                            start=True, stop=False, skip_group_check=True,
                        )
                    for j in jlist:
                        s = slice_of(j)
                        nc.tensor.matmul(
                            pss[j][:],
                            bcl_t[:, :, j * 128:(j + 1) * 128],
                            st_all[s][:, :, tsl],
                            start=False, stop=False,
                            perf_mode=DR, skip_group_check=True,
                        )
                    for j in jlist:
                        blk = j - p * WBLK
                        for i in range(1, NKT - 2):
                            wsl = (w_c[p][3][:, i - 12, blk * 128:(blk + 1) * 128]
                                   if i >= 12 else
                                   w_c[p][i // 4][:, i % 4,
                                                  blk * 128:(blk + 1) * 128])
                            nc.tensor.matmul(
                                pss[j][:], wsl, xb_i(i),
                                start=False, stop=False,
                                skip_group_check=True,
                            )
                        nc.tensor.matmul(
                            pss[j][:],
                            w8_c[p][:, :, blk * 128:(blk + 1) * 128],
                            x8_c[3][:, 2:4, :],
                            start=False, stop=True,
                            perf_mode=DR, skip_group_check=True,
                        )
                        o_t = op.tile([128, NT], F32, tag="o")
                        nc.vector.tensor_scalar_add(o_t[:], pss[j][:],
                                                    ba_t[:, j:j + 1])
                        out_dma(
                            out=outT[j * 128:(j + 1) * 128, tsl], in_=o_t[:])

                for p in range(NPASS):
                    js = list(range(p * WBLK, (p + 1) * WBLK))
                    if tt == ntt - 1 and p == NPASS - 1:
                        # split the final sub-pass so the last outputs'
                        # DMA drains earlier
                        run_group(p, js[:4])
                        run_group(p, js[4:6], out_dma=nc.sync.dma_start)
                        run_group(p, js[6:], out_dma=nc.sync.dma_start)
                    else:
                        run_group(p, js)
                if x8_next is not None:
                    x8_c, xb_c = x8_next, xb_next
    nc.compile()
    return nc


_nc_cache = {}


def _get_program(tc_tokens=TC):
    if tc_tokens not in _nc_cache:
        _nc_cache[tc_tokens] = build_program(tc_tokens)
    return _nc_cache[tc_tokens]


def make_in_maps(x, W_qkv, bias_qkv, lora_a_q, lora_a_k, lora_a_v,
                 lora_b_q, lora_b_k, lora_b_v,
                 lora_bias_q, lora_bias_k, lora_bias_v,
                 token_lora_indices, ncores=NCORES):
    x = np.asarray(x, np.float32)
    idx = np.asarray(token_lora_indices).astype(np.int64)
    tc_tokens = x.shape[0] // ncores
    ntt = tc_tokens // NT

    # W: (NPASS, 128, NKT, WBLK*128); [pi, p, i, m] = W_qkv[pi*1024+m, i*128+p]
    wd = np.ascontiguousarray(
        np.asarray(W_qkv, np.float32).reshape(NPASS, WBLK * 128, NKT, 128)
        .transpose(0, 3, 2, 1)).astype(BF16NP)
    w8d_arr = np.ascontiguousarray(
        np.asarray(W_qkv, np.float32).reshape(NPASS, WBLK * 128, NKT, 128)
        [:, :, NKT - 2:, :].transpose(0, 3, 2, 1)).astype(E4NP)
    # A8: (128, NKT, 384); [p, i, m] = 8 * A_stack[m, i*128+p]
    a_stack = np.concatenate([
        np.asarray(lora_a_q, np.float32).reshape(L * R, D),
        np.asarray(lora_a_k, np.float32).reshape(L * R, D),
        np.asarray(lora_a_v, np.float32).reshape(L * R, D)], axis=0)
    a8d = np.ascontiguousarray(
        (a_stack * 8.0).reshape(384, NKT, 128).transpose(2, 1, 0)).astype(E4NP)
    # bcl: (128, 2, O); [:,0,:] = 8*B^T rows (l*R+r), [:8,1,:] = 8*lora_bias
    bcomb = np.concatenate([
        np.asarray(lora_b_q, np.float32).transpose(0, 2, 1).reshape(L * R, QS),
        np.asarray(lora_b_k, np.float32).transpose(0, 2, 1).reshape(L * R, KVS),
        np.asarray(lora_b_v, np.float32).transpose(0, 2, 1).reshape(L * R, KVS)],
        axis=1)
    biasL = np.concatenate([
        np.asarray(lora_bias_q, np.float32),
        np.asarray(lora_bias_k, np.float32),
        np.asarray(lora_bias_v, np.float32)], axis=1)
    bcld = np.zeros((128, 2, O), np.float32)
    bcld[:, 0, :] = bcomb * 8.0
    bcld[:8, 1, :] = biasL * 8.0
    bcld = bcld.astype(E4NP)
    bad = np.ascontiguousarray(
        np.asarray(bias_qkv, np.float32).reshape(NBLK, 128).T)
    lane = np.arange(128) // R

    in_maps = []
    for c in range(ncores):
        sl = slice(c * tc_tokens, (c + 1) * tc_tokens)
        xc = x[sl]
        # (ntt, 128, NKT, NT); [tt, p, i, n] = x[tt*NT+n, i*128+p]
        xr = np.ascontiguousarray(
            xc.reshape(ntt, NT, NKT, 128).transpose(0, 3, 2, 1))
        idx_c = idx[sl]
        in_maps.append({
            "x8d": xr.astype(E4NP),
            "xbd": xr.astype(BF16NP),
            "wd": wd,
            "w8d": w8d_arr,
            "a8d": a8d,
            "bcld": bcld,
            "m8d": np.where(idx_c[None, :] == lane[:, None],
                            np.float32(1 / 64), np.float32(0)).astype(E4NP),
            "oh8d": np.where(idx_c[None, :] == np.arange(L)[:, None],
                             np.float32(0.125), np.float32(0)).astype(E4NP),
            "bad": bad,
        })
    return in_maps, tc_tokens


def kernel(x, W_qkv, bias_qkv, lora_a_q, lora_a_k, lora_a_v,
           lora_b_q, lora_b_k, lora_b_v,
           lora_bias_q, lora_bias_k, lora_bias_v,
           token_lora_indices):
    in_maps, tc_tokens = make_in_maps(
        x, W_qkv, bias_qkv, lora_a_q, lora_a_k, lora_a_v,
        lora_b_q, lora_b_k, lora_b_v,
        lora_bias_q, lora_bias_k, lora_bias_v, token_lora_indices)
    nc = _get_program(tc_tokens)
    res = None
    for attempt in range(3):
        try:
            res = run_bass_kernel_spmd(nc, in_maps, list(range(NCORES)))
            break
        except Exception:
            if attempt == 2:
                raise
    out = np.empty((tc_tokens * NCORES, O), np.float32)
    for c in range(NCORES):
        out[c * tc_tokens:(c + 1) * tc_tokens] = res.results[c]["outT"].T
    return out


# revision 26
# speedup vs baseline: 1.0714x; 1.0019x over previous
"""MergedQKVParallelLinearWithLora on 8 TRN2 NeuronCores.

Token-parallel across the 8 cores: each core computes 4096 tokens of the
full (T=32768, O=3072) output. Per core, per 512-token tile:

  shrink:  s~ = (1/64) * mask * (x8 @ (8*A)^T)   fp8e4 DoubleRow, K=2048
  main:    base bf16 matmul k-tile 0 (start=True)
         + lora expand+bias (one fp8 DoubleRow matmul, K=256: subtile0 =
           s~ rows vs 8*B columns, subtile1 = oh/8 rows vs 8*lora_bias
           rows, zero padded) accumulating into the same PSUM bank
         + base bf16 matmuls k-tiles 1..15
         + per-channel bias (DVE tensor_scalar_add at PSUM eviction)

W (bf16, 96KB/partition) stays fully SBUF-resident, so x streams exactly
once per dtype (bf16 for the base matmul, fp8 pair-layout for the shrink).
Tile-0 inputs and W are loaded through separate 4-ktile chunk tiles so the
first consumers wait on one chunk DMA, not the whole tensor. All
reshapes/transposes/dtype casts are host-side; fp8 scales are chosen so
every operand sits in e4m3's normal range (A,B,bias *8; s~ /8; the product
scales cancel).
"""

import numpy as np
import ml_dtypes

import concourse.mybir as mybir
import concourse.tile as tile
from concourse import bacc
from concourse.bass_utils import run_bass_kernel_spmd

T, D, QS, KVS, L, R = 32768, 2048, 2048, 512, 8, 16
O = QS + 2 * KVS          # 3072
NCORES = 8
TC = T // NCORES          # 4096 tokens per core
NT = 512                  # tokens per tile (matmul moving dim)
NTT = TC // NT            # 8 token tiles
NKT = D // 128            # 16 contraction k-tiles
NBLK = O // 128           # 24 output-channel blocks
WBLK = 8                  # blocks per sub-pass (3 sub-passes)
NPASS = NBLK // WBLK
NQ = 4                    # k-chunks per 16-ktile load (4 ktiles each)

F32 = mybir.dt.float32
BF16 = mybir.dt.bfloat16
FP8 = mybir.dt.float8e4
DR = mybir.MatmulPerfMode.DoubleRow
BF16NP = ml_dtypes.bfloat16
E4NP = ml_dtypes.float8_e4m3


def build_program(tc_tokens=TC):
    ntt = tc_tokens // NT
    nc = bacc.Bacc(None, target_bir_lowering=False, debug=False)

    x8d = nc.dram_tensor("x8d", [ntt, 128, NKT, NT], FP8, kind="ExternalInput")
    xbd = nc.dram_tensor("xbd", [ntt, 128, NKT, NT], BF16, kind="ExternalInput")
    wd = nc.dram_tensor("wd", [NPASS, 128, NKT, WBLK * 128], BF16,
                        kind="ExternalInput")
    a8d = nc.dram_tensor("a8d", [128, NKT, 384], FP8, kind="ExternalInput")
    w8d = nc.dram_tensor("w8d", [NPASS, 128, 2, WBLK * 128], FP8,
                         kind="ExternalInput")
    bcld = nc.dram_tensor("bcld", [128, 2, O], FP8, kind="ExternalInput")
    m8d = nc.dram_tensor("m8d", [128, tc_tokens], FP8, kind="ExternalInput")
    oh8d = nc.dram_tensor("oh8d", [8, tc_tokens], FP8, kind="ExternalInput")
    bad = nc.dram_tensor("bad", [128, NBLK], F32, kind="ExternalInput")
    outT = nc.dram_tensor("outT", [O, tc_tokens], F32, kind="ExternalOutput")

    def slice_of(j):
        return 0 if j < QS // 128 else (1 if j < (QS + KVS) // 128 else 2)

    with tile.TileContext(nc) as tc:
        with tc.tile_pool(name="const", bufs=1) as const, \
             tc.tile_pool(name="x8p", bufs=2) as x8p, \
             tc.tile_pool(name="xbp", bufs=2) as xbp, \
             tc.tile_pool(name="psm", bufs=8, space="PSUM") as psm, \
             tc.tile_pool(name="op", bufs=4) as op:
            # A8 and W as separate 4-ktile chunk tiles: consumers then wait
            # on one chunk's DMA instead of the full tensor
            a8_c = [const.tile([128, 4, 384], FP8, tag=f"a8_{q}",
                               name=f"a8_{q}") for q in range(NQ)]
            w_c = [[const.tile([128, 4 if q < 3 else 2, WBLK * 128], BF16,
                               tag=f"w{p}_{q}", name=f"w{p}_{q}")
                    for q in range(NQ)] for p in range(NPASS)]
            w8_c = [const.tile([128, 2, WBLK * 128], FP8, tag=f"w8_{p}",
                               name=f"w8_{p}") for p in range(NPASS)]
            dum_t = const.tile([128, NT], BF16, tag="dum")
            bcl_t = const.tile([128, 2, O], FP8, tag="bcl")
            m8_t = const.tile([128, tc_tokens], FP8, tag="m8")
            ba_t = const.tile([128, NBLK], F32, tag="ba")
            st_all = [const.tile([128, 2, tc_tokens], FP8, tag=f"st{s}",
                                 name=f"st{s}") for s in range(3)]
            def load_x8(tt):
                ts = []
                for q in range(NQ):
                    t = x8p.tile([128, 4, NT], FP8, tag=f"x8_{q}",
                                 name=f"x8_{tt}_{q}")
                    nc.sync.dma_start(out=t[:], in_=x8d[tt][:, 4 * q:4 * q + 4, :])
                    ts.append(t)
                return ts

            def load_xb(tt):
                # k-tiles 14,15 ride the fp8 stream instead; chunk 3 only
                # carries 12,13
                ts = []
                for q in range(NQ - 1):
                    t = xbp.tile([128, 4, NT], BF16, tag=f"xb_{q}",
                                 name=f"xb_{tt}_{q}")
                    nc.sync.dma_start(out=t[:], in_=xbd[tt][:, 4 * q:4 * q + 4, :])
                    ts.append(t)
                t = xbp.tile([128, 2, NT], BF16, tag="xb_3", name=f"xb_{tt}_3")
                nc.sync.dma_start(out=t[:], in_=xbd[tt][:, 12:14, :])
                ts.append(t)
                return ts

            # PE warm-up: dummy matmuls keep the HAM activity monitor busy
            # during the initial DMA latency so the first real matmuls run
            # at full clock
            nc.any.memset(dum_t[:], 0)
            wps = psm.tile([128, NT], F32, tag="ps", name="warm")
            for _ in range(8):
                nc.tensor.matmul(wps[:], dum_t[:, 0:128], dum_t[:],
                                 start=True, stop=True, skip_group_check=True)

            # ---- startup DMAs: everything on the sync queue, in strict
            # first-use order, so the critical first chunks aren't racing
            # other loads for the shared DMA engines
            def load_w(p):
                for q in range(3):
                    nc.sync.dma_start(out=w_c[p][q][:],
                                      in_=wd[p][:, 4 * q:4 * q + 4, :])
                nc.sync.dma_start(out=w_c[p][3][:], in_=wd[p][:, 12:14, :])
                nc.sync.dma_start(out=w8_c[p][:], in_=w8d[p])

            x8_c, xb_c = [], []
            for q in range(NQ):
                t = x8p.tile([128, 4, NT], FP8, tag=f"x8_{q}", name=f"x8_0_{q}")
                nc.sync.dma_start(out=t[:], in_=x8d[0][:, 4 * q:4 * q + 4, :])
                x8_c.append(t)
                nc.sync.dma_start(out=a8_c[q][:],
                                  in_=a8d[:, 4 * q:4 * q + 4, :])
            nc.sync.dma_start(out=m8_t[:], in_=m8d[:])
            nc.sync.dma_start(out=w_c[0][0][:], in_=wd[0][:, 0:4, :])
            t = xbp.tile([128, 4, NT], BF16, tag="xb_0", name="xb_0_0")
            nc.sync.dma_start(out=t[:], in_=xbd[0][:, 0:4, :])
            xb_c.append(t)
            nc.any.memset(st_all[0][:, 1, :], 0)
            nc.sync.dma_start(out=st_all[0][0:8, 1, :], in_=oh8d[:])
            nc.sync.dma_start(out=bcl_t[:], in_=bcld[:])
            for q in range(1, NQ - 1):
                nc.sync.dma_start(out=w_c[0][q][:],
                                  in_=wd[0][:, 4 * q:4 * q + 4, :])
                t = xbp.tile([128, 4, NT], BF16, tag=f"xb_{q}", name=f"xb_0_{q}")
                nc.sync.dma_start(out=t[:], in_=xbd[0][:, 4 * q:4 * q + 4, :])
                xb_c.append(t)
            nc.sync.dma_start(out=w_c[0][3][:], in_=wd[0][:, 12:14, :])
            t = xbp.tile([128, 2, NT], BF16, tag="xb_3", name="xb_0_3")
            nc.sync.dma_start(out=t[:], in_=xbd[0][:, 12:14, :])
            xb_c.append(t)
            nc.sync.dma_start(out=w8_c[0][:], in_=w8d[0])
            load_w(1)
            nc.sync.dma_start(out=ba_t[:], in_=bad[:])
            for s in (1, 2):
                nc.any.memset(st_all[s][:, 1, :], 0)
                nc.sync.dma_start(out=st_all[s][0:8, 1, :], in_=oh8d[:])
            load_w(2)

            for tt in range(ntt):
                tsl = slice(tt * NT, (tt + 1) * NT)
                # ---- shrink: fp8 DoubleRow, all 3 slices ----
                for s in range(3):
                    ps = psm.tile([128, NT], F32, tag="ps", name=f"shr{s}_{tt}")
                    for k in range(NKT // 2):
                        nc.tensor.matmul(
                            ps[:],
                            a8_c[k // 2][:, 2 * (k % 2):2 * (k % 2) + 2,
                                         s * 128:(s + 1) * 128],
                            x8_c[k // 2][:, 2 * (k % 2):2 * (k % 2) + 2, :],
                            start=(k == 0), stop=(k == NKT // 2 - 1),
                            perf_mode=DR, skip_group_check=True,
                        )
                    nc.vector.tensor_mul(st_all[s][:, 0, tsl], ps[:], m8_t[:, tsl])

                # prefetch next token tile while mains chew
                if tt + 1 < ntt:
                    x8_next = load_x8(tt + 1)
                    xb_next = load_xb(tt + 1)
                else:
                    x8_next = xb_next = None

                # ---- main: sub-passes of channel blocks. Each bank opens
                # with a base bf16 matmul (start=True), the fp8 DoubleRow
                # lora-expand accumulates behind it, then the remaining
                # base k-tiles drain the bank.
                def xb_i(i):
                    return xb_c[3][:, i - 12, :] if i >= 12 else \
                        xb_c[i // 4][:, i % 4, :]

                def run_group(p, jlist, out_dma=nc.gpsimd.dma_start):
                    pss = {}
                    for j in jlist:
                        blk = j - p * WBLK
                        ps = psm.tile([128, NT], F32, tag="ps", name=f"ps{j}_{tt}")
                        pss[j] = ps
                        nc.tensor.matmul(
                            ps[:],
                            w_c[p][0][:, 0, blk * 128:(blk + 1) * 128],
                            xb_i(0),
                            start=True, stop=False, skip_group_check=True,
                        )
                    for j in jlist:
                        s = slice_of(j)
                        nc.tensor.matmul(
                            pss[j][:],
                            bcl_t[:, :, j * 128:(j + 1) * 128],
                            st_all[s][:, :, tsl],
                            start=False, stop=False,
                            perf_mode=DR, skip_group_check=True,
                        )
                    for j in jlist:
                        blk = j - p * WBLK
                        for i in range(1, NKT - 2):
                            wsl = (w_c[p][3][:, i - 12, blk * 128:(blk + 1) * 128]
                                   if i >= 12 else
                                   w_c[p][i // 4][:, i % 4,
                                                  blk * 128:(blk + 1) * 128])
                            nc.tensor.matmul(
                                pss[j][:], wsl, xb_i(i),
                                start=False, stop=False,
                                skip_group_check=True,
                            )
                        nc.tensor.matmul(
                            pss[j][:],
                            w8_c[p][:, :, blk * 128:(blk + 1) * 128],
                            x8_c[3][:, 2:4, :],
                            start=False, stop=True,
                            perf_mode=DR, skip_group_check=True,
                        )
                        o_t = op.tile([128, NT], F32, tag="o")
                        nc.vector.tensor_scalar_add(o_t[:], pss[j][:],
                                                    ba_t[:, j:j + 1])
                        out_dma(
                            out=outT[j * 128:(j + 1) * 128, tsl], in_=o_t[:])

                for p in range(NPASS):
                    js = list(range(p * WBLK, (p + 1) * WBLK))
                    if tt == ntt - 1 and p == NPASS - 1:
                        # split the final sub-pass so the last outputs'
                        # DMA drains earlier
                        run_group(p, js[:4])
                        run_group(p, js[4:6], out_dma=nc.sync.dma_start)
                        run_group(p, js[6:], out_dma=nc.sync.dma_start)
                    else:
                        run_group(p, js)
                if x8_next is not None:
                    x8_c, xb_c = x8_next, xb_next
    nc.compile()
    return nc


_nc_cache = {}


def _get_program(tc_tokens=TC):
    if tc_tokens not in _nc_cache:
        _nc_cache[tc_tokens] = build_program(tc_tokens)
    return _nc_cache[tc_tokens]


def make_in_maps(x, W_qkv, bias_qkv, lora_a_q, lora_a_k, lora_a_v,
                 lora_b_q, lora_b_k, lora_b_v,
                 lora_bias_q, lora_bias_k, lora_bias_v,
                 token_lora_indices, ncores=NCORES):
    x = np.asarray(x, np.float32)
    idx = np.asarray(token_lora_indices).astype(np.int64)
    tc_tokens = x.shape[0] // ncores
    ntt = tc_tokens // NT

    # W: (NPASS, 128, NKT, WBLK*128); [pi, p, i, m] = W_qkv[pi*1024+m, i*128+p]
    wd = np.ascontiguousarray(
        np.asarray(W_qkv, np.float32).reshape(NPASS, WBLK * 128, NKT, 128)
        .transpose(0, 3, 2, 1)).astype(BF16NP)
    w8d_arr = np.ascontiguousarray(
        np.asarray(W_qkv, np.float32).reshape(NPASS, WBLK * 128, NKT, 128)
        [:, :, NKT - 2:, :].transpose(0, 3, 2, 1)).astype(E4NP)
    # A8: (128, NKT, 384); [p, i, m] = 8 * A_stack[m, i*128+p]
    a_stack = np.concatenate([
        np.asarray(lora_a_q, np.float32).reshape(L * R, D),
        np.asarray(lora_a_k, np.float32).reshape(L * R, D),
        np.asarray(lora_a_v, np.float32).reshape(L * R, D)], axis=0)
    a8d = np.ascontiguousarray(
        (a_stack * 8.0).reshape(384, NKT, 128).transpose(2, 1, 0)).astype(E4NP)
    # bcl: (128, 2, O); [:,0,:] = 8*B^T rows (l*R+r), [:8,1,:] = 8*lora_bias
    bcomb = np.concatenate([
        np.asarray(lora_b_q, np.float32).transpose(0, 2, 1).reshape(L * R, QS),
        np.asarray(lora_b_k, np.float32).transpose(0, 2, 1).reshape(L * R, KVS),
        np.asarray(lora_b_v, np.float32).transpose(0, 2, 1).reshape(L * R, KVS)],
        axis=1)
    biasL = np.concatenate([
        np.asarray(lora_bias_q, np.float32),
        np.asarray(lora_bias_k, np.float32),
        np.asarray(lora_bias_v, np.float32)], axis=1)
    bcld = np.zeros((128, 2, O), np.float32)
    bcld[:, 0, :] = bcomb * 8.0
    bcld[:8, 1, :] = biasL * 8.0
    bcld = bcld.astype(E4NP)
    bad = np.ascontiguousarray(
        np.asarray(bias_qkv, np.float32).reshape(NBLK, 128).T)
    lane = np.arange(128) // R

    in_maps = []
    for c in range(ncores):
        sl = slice(c * tc_tokens, (c + 1) * tc_tokens)
        xc = x[sl]
        # (ntt, 128, NKT, NT); [tt, p, i, n] = x[tt*NT+n, i*128+p]
        xr = np.ascontiguousarray(
            xc.reshape(ntt, NT, NKT, 128).transpose(0, 3, 2, 1))
        idx_c = idx[sl]
        in_maps.append({
            "x8d": xr.astype(E4NP),
            "xbd": xr.astype(BF16NP),
            "wd": wd,
            "w8d": w8d_arr,
            "a8d": a8d,
            "bcld": bcld,
            "m8d": np.where(idx_c[None, :] == lane[:, None],
                            np.float32(1 / 64), np.float32(0)).astype(E4NP),
            "oh8d": np.where(idx_c[None, :] == np.arange(L)[:, None],
                             np.float32(0.125), np.float32(0)).astype(E4NP),
            "bad": bad,
        })
    return in_maps, tc_tokens


def kernel(x, W_qkv, bias_qkv, lora_a_q, lora_a_k, lora_a_v,
           lora_b_q, lora_b_k, lora_b_v,
           lora_bias_q, lora_bias_k, lora_bias_v,
           token_lora_indices):
    in_maps, tc_tokens = make_in_maps(
        x, W_qkv, bias_qkv, lora_a_q, lora_a_k, lora_a_v,
        lora_b_q, lora_b_k, lora_b_v,
        lora_bias_q, lora_bias_k, lora_bias_v, token_lora_indices)
    nc = _get_program(tc_tokens)
    res = None
    for attempt in range(3):
        try:
            res = run_bass_kernel_spmd(nc, in_maps, list(range(NCORES)))
            break
        except Exception:
            if attempt == 2:
                raise
    out = np.empty((tc_tokens * NCORES, O), np.float32)
    for c in range(NCORES):
        out[c * tc_tokens:(c + 1) * tc_tokens] = res.results[c]["outT"].T
    return out
